# revision 33
# baseline (speedup 1.0000x reference)
"""Bass/Tile TRN2 kernel: 16-head MHA (B=2, T=2048, D=1024, H=64) on 8 NeuronCores.

Sharding: 8-way tensor parallel over heads — core c computes heads {2c, 2c+1}
for BOTH batches. Output ownership: within every (batch, 512-row tq block),
core c owns the 64 rows [c*64, (c+1)*64). After each block's attention one
small (128KB) AllToAll exchanges head-shards for row-shards, so every
collective except the last overlaps later attention blocks, and the output
projection for a PAIR of blocks (2x64 owned rows = 128 partitions) runs as
PE filler work.

Per-core pipeline (bf16 into the PE, fp32 PSUM accumulation):
  - ~200 tiny warm-up matmuls at t=0 trip the PE HAM clock gate to 2.4 GHz
    before the first projection.
  - DMA order: k0,q0 first so scores start ~15us in; k/v interleaved next at
    the rate the attention pipeline consumes them; wp before the first
    output-projection filler is needed.
  - QKV projections: 8x [128,128]x[128,512] accumulating matmuls per block.
  - Scores S^T[tk, tq] = K^T.T @ Q^T per head ([64,128] stationary, auto
    row-group packing); 1/sqrt(H) folded into Wq/bq on host.
  - exp on ScalarE straight out of PSUM in 1024-wide ACTIVATEs; the two
    heads' score PSUM buffers alternate so ACT never waits on score matmuls.
    Attention is emitted as ONE flat software pipeline over all 8 (b, tq)
    blocks: the next block's scores/exps are emitted before the previous
    block's normalize, so ACT stays saturated across block boundaries.
  - PV matmul with a ones-augmented V (stationary col 64 = ones) so row 64 of
    the PV accumulator is the softmax denominator for free.
  - Normalize: pv evacuated by DVE (frees the PSUM bank), denominator row
    PE-transposed to [128,4] for a cheap DVE reciprocal, broadcast back via a
    1-row outer-product matmul, multiply, stage into the block's AllToAll
    buffer as [8 dest cores x 128 headdim, 64 tq].
  - PE idle slots inside the ACT-bound phase are filled with the next batch's
    projections and completed pairs' output projections.
Host does layout-only prep (transpose, bf16 cast, weight slicing) and
scatters the 8 cores' per-block 64-row output slices.
"""

import sys
from contextlib import ExitStack

import numpy as np

sys.path.insert(0, "/opt/trn_rl_repo")

import ml_dtypes  # noqa: E402

BF16 = ml_dtypes.bfloat16

B, T, D = 2, 2048, 1024
N_HEADS, H = 16, 64
NCORES = 8
GROUPS = [[0, 1, 2, 3, 4, 5, 6, 7]]
NLOC = 2            # heads per core
TQB = 512           # attention tq block
NTQB = T // TQB     # 4
TKC = 128           # tk chunk
NTKC = T // TKC     # 16
RG = 2              # tk chunks per exp group
NG = NTKC // RG     # 8 groups per block
DC = 128            # d chunk
NDC = D // DC       # 8
XB = 512            # x-load column block
OWN = 64            # tq rows per core per block
VA = 128            # V_aug stationary width: [V(64) | ones(1) | junk(63)]
NW = NLOC * H       # 128 projection width per core
WARMUP = 200        # HAM pre-warm matmul count (0 = off)
DEBUG_AN = False    # dump normalized attention tiles to a debug output

_CACHE = {}


def _legalize_waits(bir_bytes):
    """This toolchain's walrus accepts at most ONE semaphore wait per
    instruction ("Too many sync wait commands"). Tile's sem assignment emits
    several. Hoist all but one wait of each instruction onto same-engine NoOps
    inserted immediately before it (engines execute their stream in order, so
    waiting earlier on the same engine is equivalent)."""
    import json

    j = json.loads(bir_bytes)
    ctr = 0
    for fn in j["functions"]:
        for blk in fn["blocks"]:
            out = []
            for ins in blk["instructions"]:
                si = ins.get("sync_info")
                waits = (si or {}).get("on_wait") or []
                if len(waits) > 1:
                    for w in waits[:-1]:
                        ctr += 1
                        out.append(
                            {
                                "engine": ins["engine"],
                                "ins": [],
                                "outs": [],
                                "name": f"waitfix-{ctr}",
                                "opcode": "NoOp",
                                "sync_info": {"on_wait": [w], "on_update": []},
                            }
                        )
                    si["on_wait"] = [waits[-1]]
                out.append(ins)
            blk["instructions"] = out
    return json.dumps(j).encode()


def _build():
    import concourse.bass as bass
    import concourse.mybir as mybir
    import concourse.tile as tile

    f32 = mybir.dt.float32
    bf16 = mybir.dt.bfloat16
    AF = mybir.ActivationFunctionType
    ALU = mybir.AluOpType

    nc = bass.Bass(
        "TRN2", target_bir_lowering=False, debug=False, num_devices=NCORES
    )

    # activations/weights arrive pre-arranged on host into the exact SBUF
    # layout ([partition, ...] contiguous) so every load is a 1:1 DMA with
    # 2KB+ lines and 128 descriptors.
    qT = [nc.dram_tensor(f"qT{b}", [128, NTQB * NDC * XB], bf16, kind="ExternalInput") for b in range(B)]
    kT = [nc.dram_tensor(f"kT{b}", [128, NTQB * NDC * XB], bf16, kind="ExternalInput") for b in range(B)]
    vT = [nc.dram_tensor(f"vT{b}", [128, NTQB * NDC * XB], bf16, kind="ExternalInput") for b in range(B)]
    wq = nc.dram_tensor("wq", [128, NDC * NW], bf16, kind="ExternalInput")
    wk = nc.dram_tensor("wk", [128, NDC * NW], bf16, kind="ExternalInput")
    wv = nc.dram_tensor("wv", [128, NDC * NW], bf16, kind="ExternalInput")
    wp = nc.dram_tensor("wp", [128, (N_HEADS * H // 128) * D], bf16, kind="ExternalInput")
    bq = nc.dram_tensor("bq", [128, 1], f32, kind="ExternalInput")
    bk = nc.dram_tensor("bk", [128, 1], f32, kind="ExternalInput")
    bv = nc.dram_tensor("bv", [128, 1], f32, kind="ExternalInput")
    bp = nc.dram_tensor("bp", [128, D], f32, kind="ExternalInput")
    ident = nc.dram_tensor("ident", [128, 128], bf16, kind="ExternalInput")
    identf = nc.dram_tensor("identf", [128, 128], f32, kind="ExternalInput")
    # rows: (b, pair, i) with i in [0,128): j = 2*pair + i//64, own-row i%64
    out = nc.dram_tensor("out", [B * 2 * 128, D], f32, kind="ExternalOutput")
    dbg = (
        nc.dram_tensor("dbg", [B * NTQB * NLOC * H, TQB], bf16, kind="ExternalOutput")
        if DEBUG_AN
        else None
    )
    dbg2 = (
        nc.dram_tensor("dbg2", [4 * 128, NCORES * 128], bf16, kind="ExternalOutput")
        if DEBUG_AN
        else None
    )

    with tile.TileContext(nc) as tc, ExitStack() as ctx:
        p_const = ctx.enter_context(tc.tile_pool(name="const", bufs=1))
        p_x = ctx.enter_context(tc.tile_pool(name="x", bufs=1))
        p_qk = ctx.enter_context(tc.tile_pool(name="qk", bufs=2))
        p_va = ctx.enter_context(tc.tile_pool(name="va", bufs=2))
        p_pt = ctx.enter_context(tc.tile_pool(name="pt", bufs=3))
        p_a = ctx.enter_context(tc.tile_pool(name="a", bufs=2))
        p_o = ctx.enter_context(tc.tile_pool(name="o", bufs=2))
        p_dram = ctx.enter_context(tc.tile_pool(name="dram", bufs=1, space="DRAM"))

        ps_ss = ctx.enter_context(tc.tile_pool(name="ps_ss", bufs=1, space="PSUM"))
        ps_pv = ctx.enter_context(tc.tile_pool(name="ps_pv", bufs=1, space="PSUM"))
        ps_mm = ctx.enter_context(tc.tile_pool(name="ps_mm", bufs=2, space="PSUM"))

        # ---- constant tiles -------------------------------------------------
        wq_sb = p_const.tile([128, NDC * NW], bf16)
        wk_sb = p_const.tile([128, NDC * NW], bf16)
        wv_sb = p_const.tile([128, NDC * NW], bf16)
        wp_sb = p_const.tile([128, (N_HEADS * H // 128) * D], bf16)
        bq_sb = p_const.tile([128, 1], f32)
        bk_sb = p_const.tile([128, 1], f32)
        bv_sb = p_const.tile([128, 1], f32)
        bp_sb = p_const.tile([128, D], f32)
        id_sb = p_const.tile([128, 128], bf16)
        idf_sb = p_const.tile([128, 128], f32)

        # warm the ACT exp table while everything else is still loading
        warm = p_const.tile([1, 8], bf16)
        nc.vector.memset(warm[:], 0.0)
        nc.scalar.activation(warm[:], warm[:], AF.Exp)

        # ones tiles: f32 (K=1 transpose "identity") and bf16 (broadcast
        # outer-product column, 1 col/cycle instead of fp32's 1/2)
        onesf = p_const.tile([128, H], f32)
        nc.vector.memset(onesf[:], 1.0)
        onesb = p_const.tile([128, H], bf16)
        nc.vector.memset(onesb[:], 1.0)

        # ---- HAM pre-warm: keep the PE busy from t~0 so the clock gate is
        # released (1.2 -> 2.4 GHz) before the first real projection matmul.
        if WARMUP:
            wtile = p_const.tile([128, 64], bf16)
            nc.vector.memset(wtile[:], 0.0)
            wps = ps_mm.tile([64, 64], f32, name="wps", tag="mm")
            for _ in range(WARMUP):
                nc.tensor.matmul(wps[:], lhsT=wtile[:, 0:64], rhs=wtile[:], start=True, stop=True)

        BLOCKS = [(b, j) for b in range(B) for j in range(NTQB)]
        # Pair-level exchange buffers: [8 src cores x 128 headdim,
        # (half, own-64)] per (batch, pair-of-blocks). 4 collectives of 256KB
        # each — the mesh AllToAll has ~15-30us fixed cost per op, so
        # per-block (8x128KB) exchanges serialize into the critical path.
        # NOTE: untagged tiles in one pool share a single slot ring — every
        # DRAM tile needs its own tag or all collectives alias one buffer.
        PAIRS = [(b, pr) for b in range(B) for pr in range(2)]
        a2a_in = {
            pp: p_dram.tile(
                [NCORES * NW, 2 * OWN], bf16,
                name=f"a2ai{pp[0]}{pp[1]}", tag=f"a2ai{pp[0]}{pp[1]}",
            )
            for pp in PAIRS
        }
        a2a_out = {
            pp: p_dram.tile(
                [NCORES * NW, 2 * OWN], bf16,
                name=f"a2ao{pp[0]}{pp[1]}", tag=f"a2ao{pp[0]}{pp[1]}",
            )
            for pp in PAIRS
        }

        # ---- x loads: per (tensor, tb) contiguous 8KB-line chunks -----------
        CW = NDC * XB  # 4096 cols per tb chunk
        exts = {"v": vT, "k": kT, "q": qT}
        xs = {0: {}, 1: {}}

        def load_x_one(b, tname, tb):
            # tb%2 tag ring: tb=2,3 reuse tb=0,1's buffers (WAR on the proj
            # that consumed them) — halves x SBUF footprint to 48KB.
            t_ = p_x.tile(
                [128, CW], bf16, name=f"x{tname}{b}{tb}", tag=f"x{tname}{tb % 2}"
            )
            nc.sync.dma_start(t_[:], exts[tname][b][:, tb * CW : (tb + 1) * CW])
            xs[b][(tname, tb)] = t_

        def load_startup():
            """Loads in consumption order. b=0: k0,q0 first (scores start as
            soon as K0/Q0 project), then k/v interleaved at the rate block
            (0,0) consumes chunks, then q1-3 (needed from block (0,1)).
            b=1 x-loads reuse b=0's buffers (same tag -> WAR on b0 proj).
            wp (2MB) before b1 q-loads: first outproj filler fires ~85us in."""
            nc.sync.dma_start(id_sb[:], ident[:])
            nc.sync.dma_start(wk_sb[:], wk[:])
            nc.sync.dma_start(bk_sb[:], bk[:])
            load_x_one(0, "k", 0)
            nc.sync.dma_start(wq_sb[:], wq[:])
            nc.sync.dma_start(bq_sb[:], bq[:])
            load_x_one(0, "q", 0)
            load_x_one(0, "k", 1)
            nc.sync.dma_start(wv_sb[:], wv[:])
            nc.sync.dma_start(bv_sb[:], bv[:])
            load_x_one(0, "v", 0)
            load_x_one(0, "v", 1)
            load_x_one(0, "k", 2)
            load_x_one(0, "v", 2)
            load_x_one(0, "k", 3)
            load_x_one(0, "q", 1)
            load_x_one(0, "v", 3)
            load_x_one(0, "q", 2)
            load_x_one(0, "q", 3)
            nc.sync.dma_start(idf_sb[:], identf[:])
            nc.sync.dma_start(bp_sb[:], bp[:])
            for tb in range(NTQB):
                load_x_one(1, "v", tb)
                load_x_one(1, "k", tb)
            nc.sync.dma_start(wp_sb[:], wp[:])
            for tb in range(NTQB):
                load_x_one(1, "q", tb)

        # ---- projection pieces (emitted inline or as PE fillers) ------------
        def proj_block(w_sb, b_sb, xt, dst, dcol, b, tag):
            ps = ps_mm.tile([128, XB], f32, name=f"mm{tag}{b}{dcol}", tag="mm")
            for dc in range(NDC):
                nc.tensor.matmul(
                    ps[:],
                    lhsT=w_sb[:, dc * NW : (dc + 1) * NW],
                    rhs=xt[:, dc * XB : (dc + 1) * XB],
                    start=(dc == 0),
                    stop=(dc == NDC - 1),
                )
            nc.vector.tensor_scalar(
                dst[:, dcol * XB : (dcol + 1) * XB], ps[:], b_sb[:, 0:1], None,
                ALU.add,
            )

        def va_piece(b, vt, va, i0, n):
            """Transpose tk-chunks [i0, i0+n) of vt into va."""
            for i in range(i0, i0 + n):
                pst = ps_mm.tile([128, 128], bf16, name=f"pst{b}{i}", tag="mm")
                nc.tensor.transpose(pst[:], vt[:, i * TKC : (i + 1) * TKC], id_sb[:])
                dst = va[:, i * NLOC * VA : (i + 1) * NLOC * VA].rearrange(
                    "p (h x) -> p h x", x=VA
                )[:, :, 0:H]
                nc.vector.tensor_copy(dst, pst[:].rearrange("p (h x) -> p h x", x=H))

        def make_proj(b):
            """Returns (qt, kt, va, pieces): pieces in pipeline-consumption
            order (K first, then V/va interleaved, Q last)."""
            vt = p_va.tile([128, T], bf16, name=f"vt{b}", tag="vt")
            va = p_va.tile([128, NTKC * NLOC * VA], bf16, name=f"va{b}", tag="va")
            qt = p_qk.tile([128, T], bf16, name=f"qt{b}", tag="qt")
            kt = p_qk.tile([128, T], bf16, name=f"kt{b}", tag="kt")
            nc.vector.memset(
                va[:].rearrange("p (i h x) -> p i h x", h=NLOC, x=VA)[
                    :, :, :, H : H + 1
                ],
                1.0,
            )
            mk = lambda w, bb, tn, dst, tb, tg: (
                lambda: proj_block(w, bb, xs[b][(tn, tb)], dst, tb, b, tg)
            )
            # consumption order matches the DMA arrival order (~1 chunk per
            # exp group): V tb / va / K tb interleaved, Q blocks last.
            pieces = [
                mk(wk_sb, bk_sb, "k", kt, 0, "k"),
                mk(wq_sb, bq_sb, "q", qt, 0, "q"),
                mk(wv_sb, bv_sb, "v", vt, 0, "v"),
                lambda: va_piece(b, vt, va, 0, 4),
                mk(wk_sb, bk_sb, "k", kt, 1, "k"),
                mk(wv_sb, bv_sb, "v", vt, 1, "v"),
                lambda: va_piece(b, vt, va, 4, 4),
                mk(wk_sb, bk_sb, "k", kt, 2, "k"),
                mk(wv_sb, bv_sb, "v", vt, 2, "v"),
                lambda: va_piece(b, vt, va, 8, 4),
                mk(wk_sb, bk_sb, "k", kt, 3, "k"),
                mk(wv_sb, bv_sb, "v", vt, 3, "v"),
                lambda: va_piece(b, vt, va, 12, 4),
                mk(wq_sb, bq_sb, "q", qt, 1, "q"),
                mk(wq_sb, bq_sb, "q", qt, 2, "q"),
                mk(wq_sb, bq_sb, "q", qt, 3, "q"),
            ]
            return qt, kt, va, pieces

        fillers = []

        def run_filler(n=1):
            for _ in range(n):
                if fillers:
                    fillers.pop(0)()

        # ---- output projection for a PAIR of blocks -------------------------
        # ats_pair[:, s*128 + half*64 : +64] holds src-core s's heads for the
        # half-th block of the pair; one [128,512] matmul group per D-half.
        def outproj_pair(b, pr, ats_pair):
            def op(dh):
                ps = ps_mm.tile([128, 512], f32, name=f"po{b}{pr}{dh}", tag="mm")
                for s in range(NCORES):
                    nc.tensor.matmul(
                        ps[:],
                        lhsT=ats_pair[:, s * 128 : (s + 1) * 128],
                        rhs=wp_sb[:, s * D + dh * 512 : s * D + (dh + 1) * 512],
                        start=(s == 0),
                        stop=(s == NCORES - 1),
                    )
                o_sb = p_o.tile([128, 512], f32, name=f"o{b}{pr}{dh}", tag="o")
                nc.vector.tensor_tensor(
                    o_sb[:], ps[:], bp_sb[:, dh * 512 : (dh + 1) * 512], ALU.add
                )
                nc.sync.dma_start(
                    out[
                        (b * 2 + pr) * 128 : (b * 2 + pr + 1) * 128,
                        dh * 512 : (dh + 1) * 512,
                    ],
                    o_sb[:],
                )
            return [lambda dh=dh: op(dh) for dh in range(2)]

        def ats_load(b, pr, ats_pair):
            for s in range(NCORES):
                nc.sync.dma_start(
                    ats_pair[:, s * 128 : (s + 1) * 128],
                    a2a_out[(b, pr)][s * 128 : (s + 1) * 128, :],
                )

        # ---- attention pieces ----------------------------------------------
        st = {}  # live per-block state: pv tiles, pss, pt, a_sb

        def scores(bl, g, qt, kt):
            # head-major: each head's chunk pair releases together (gated by
            # its exp) and the two MMs pipeline fill-under-drain.
            b, j = bl
            for hd in range(NLOC):
                key = (bl, g % 2, hd)
                st[("ss",) + key] = ps_ss.tile(
                    [128, RG * TQB], f32,
                    name=f"pss{b}{j}{g}{hd}", tag=f"ss{hd}",
                )
                for r in range(g * RG, (g + 1) * RG):
                    nc.tensor.matmul(
                        st[("ss",) + key][:, (r % RG) * TQB : (r % RG + 1) * TQB],
                        lhsT=kt[hd * H : (hd + 1) * H, r * TKC : (r + 1) * TKC],
                        rhs=qt[hd * H : (hd + 1) * H, j * TQB : (j + 1) * TQB],
                        start=True,
                        stop=True,
                    )

        def exps(bl, g):
            b, j = bl
            for hd in range(NLOC):
                pt_t = p_pt.tile(
                    [128, RG * TQB], bf16,
                    name=f"pt{b}{j}{g}{hd}", tag=f"pt{hd}",
                )
                st[("pt", bl, g % 2, hd)] = pt_t
                nc.scalar.activation(
                    pt_t[:], st[("ss", bl, g % 2, hd)][:], AF.Exp
                )

        def pvs(bl, g, va):
            b, j = bl
            if g == 0:
                st[("pv", bl)] = [
                    ps_pv.tile([VA, TQB], f32, name=f"pv{b}{j}{hd}", tag=f"pv{hd}")
                    for hd in range(NLOC)
                ]
            pv = st[("pv", bl)]
            for hd in range(NLOC):
                for r in range(g * RG, (g + 1) * RG):
                    col0 = (r * NLOC + hd) * VA
                    nc.tensor.matmul(
                        pv[hd][:],
                        lhsT=va[:, col0 : col0 + VA],
                        rhs=st[("pt", bl, g % 2, hd)][:, (r % RG) * TQB : (r % RG + 1) * TQB],
                        start=(g == 0 and r == g * RG),
                        stop=(g == NG - 1 and r == (g + 1) * RG - 1),
                    )

        def norm_a(bl):
            """Evacuate pv -> SBUF (frees the pv PSUM banks for the next
            block's accumulation). 96 partitions so the denominator row (64)
            sits in a 32-aligned window for the DVE block-transpose."""
            b, j = bl
            pv = st.pop(("pv", bl))
            for hd in range(NLOC):
                a_sb = p_a.tile(
                    [96, TQB], f32, name=f"as{b}{j}{hd}", tag=f"as{hd}"
                )
                nc.vector.tensor_copy(a_sb[0 : H + 1, :], pv[hd][0 : H + 1, :])
                st[("as", bl, hd)] = a_sb

        def norm_b(bl, hd):
            """Reciprocal of the denominator row, all on DVE: 32x32
            block-transpose puts den[32j+r] at [r, 32j], a strided reciprocal
            hits only those lanes, a second block-transpose puts 1/den back
            as row 0; an fp32 outer-product matmul broadcasts it to H rows
            for the multiply. Then stage into the pair's AllToAll input."""
            b, j = bl
            a_sb = st.pop(("as", bl, hd))
            tr = p_a.tile([32, TQB], f32, name=f"tr{b}{j}{hd}", tag=f"tr{hd}", bufs=1)
            nc.vector.transpose(tr[:], a_sb[64:96, :])
            rc = p_a.tile([32, TQB], f32, name=f"rc{b}{j}{hd}", tag=f"rc{hd}", bufs=1)
            nc.vector.reciprocal(
                rc[:].rearrange("p (j c) -> p j c", c=32)[:, :, 0:1],
                tr[:].rearrange("p (j c) -> p j c", c=32)[:, :, 0:1],
            )
            rw = p_a.tile([32, TQB], f32, name=f"rw{b}{j}{hd}", tag=f"rw{hd}", bufs=1)
            nc.vector.transpose(rw[:], rc[:])
            rep_ps = ps_mm.tile([H, TQB], f32, name=f"rp{b}{j}{hd}", tag="mm")
            nc.tensor.matmul(
                rep_ps[:], lhsT=onesf[0:1, 0:H], rhs=rw[0:1, :], start=True, stop=True
            )
            an = p_a.tile([H, TQB], bf16, name=f"an{b}{j}{hd}", tag=f"an{hd}")
            nc.vector.tensor_tensor(an[:], a_sb[0:H, :], rep_ps[:], ALU.mult)
            if dbg is not None:
                r0 = ((b * NTQB + j) * NLOC + hd) * H
                nc.sync.dma_start(dbg[r0 : r0 + H, :], an[:])
            # stage into the pair buffer: rows c*128 + hd*64 + h, cols
            # (j%2)*64 + own-64 of tq. Iteration order (h, c, t) on both
            # sides; SBUF AP keeps the partition dim (h) first.
            half = j % 2
            dst = a2a_in[(b, j // 2)].rearrange("(c s) t -> s c t", s=NW)[
                hd * H : (hd + 1) * H, :, half * OWN : (half + 1) * OWN
            ]
            src = an[:].rearrange("h (c t) -> h c t", t=OWN)
            # SP queue: behind the x-loads (first ~75us), which is fine — the
            # collective chain is trigger-paced after that. (scalar/gpsimd
            # DMA queues corrupt this strided transfer: NaN / wrong layout.)
            nc.sync.dma_start(dst, src)

        def collective(pp):
            nc.gpsimd.collective_compute(
                "AllToAll",
                mybir.AluOpType.bypass,
                replica_groups=GROUPS,
                ins=[a2a_in[pp].opt()],
                outs=[a2a_out[pp].opt()],
            )

        # ===== main schedule =================================================
        load_startup()
        qt0, kt0, va0, pieces0 = make_proj(0)
        # inline: K0, Q0 only -- scores start as soon as they project.
        pieces0[0]()
        pieces0[1]()
        # b0 filler order = consumption order (V0/K1/va03 lead; Q1/Q2 hoisted
        # so each lands >=1 group before its block's first scores).
        fillers.extend(
            [pieces0[i] for i in (2, 4, 3, 7, 5, 6, 8, 10, 9, 13, 11, 12, 14, 15)]
        )

        qt1, kt1, va1, p1 = make_proj(1)
        # b1 consumption order matching its v/k-interleaved DMA arrival
        fillers.extend(
            [p1[i] for i in (2, 0, 3, 5, 4, 6, 8, 7, 9, 11, 10, 12, 1, 13, 14, 15)]
        )

        # EMISSION ORDER IS DATAFLOW: a filler emitted after its consumer's
        # emission silently reads stale data. Pops per (bi, g), hand-paced to
        # DMA arrival while keeping >=1 group of margin before each consumer.
        # bi=0 list: [V0, K1, va03 | - | K2, V1, va47 | V2 | K3, va811 | Q1 |
        #             V3, va1215, Q2 | -]
        POPS = {
            0: [3, 0, 3, 1, 2, 1, 3, 0],
            1: [0, 0, 0, 2, 1, 2, 1, 1],
            2: [0, 0, 0, 1, 1, 1, 1, 1],
            3: [0, 0, 0, 1, 1, 1, 1, 1],
        }
        DEF_POPS = [0, 0, 0, 0, 2, 2, 2, 2]

        qkv = {0: (qt0, kt0, va0), 1: (qt1, kt1, va1)}
        ats_pairs = {}
        for b in range(B):
            for pr in range(2):
                ats_pairs[(b, pr)] = p_a.tile(
                    [128, NCORES * 128], bf16, name=f"ats{b}{pr}", tag=f"ats{pr}"
                )

        NB = len(BLOCKS)
        scores(BLOCKS[0], 0, qt0, kt0)
        exps(BLOCKS[0], 0)
        for bi, bl in enumerate(BLOCKS):
            b, j = bl
            qt, kt, va = qkv[b]
            for g in range(NG):
                # one-group lookahead (crosses block boundaries)
                if g + 1 < NG:
                    scores(bl, g + 1, qt, kt)
                    exps(bl, g + 1)
                elif bi + 1 < NB:
                    nbl = BLOCKS[bi + 1]
                    nqt, nkt, _ = qkv[nbl[0]]
                    scores(nbl, 0, nqt, nkt)
                    exps(nbl, 0)
                if bi > 0:
                    pbl = BLOCKS[bi - 1]
                    if g == 0:
                        norm_b(pbl, 0)
                    elif g == 1:
                        norm_b(pbl, 1)
                    elif g == 2 and pbl[1] % 2 == 1:
                        collective((pbl[0], pbl[1] // 2))
                    elif g == 3 and bi in (4, 6):
                        # ats for the pair exchanged TWO blocks ago (its
                        # collective is long done -> no SP-queue stall)
                        pb, pj = BLOCKS[bi - 3]
                        pr = pj // 2
                        ats_load(pb, pr, ats_pairs[(pb, pr)])
                        fillers.extend(outproj_pair(pb, pr, ats_pairs[(pb, pr)]))
                run_filler(POPS.get(bi, DEF_POPS)[g])
                pvs(bl, g, va)
                if g == NG - 1:
                    norm_a(bl)

        # ---- tail: last block's norm + pair collectives + outprojs ---------
        last = BLOCKS[-1]
        norm_b(last, 0)
        norm_b(last, 1)
        collective((1, 1))
        # pair (1,0)'s collective was triggered two blocks ago; its outproj
        # runs while pair (1,1)'s collective is in flight.
        ats_load(1, 0, ats_pairs[(1, 0)])
        for p in outproj_pair(1, 0, ats_pairs[(1, 0)]):
            p()
        run_filler(len(fillers))
        ats_load(1, 1, ats_pairs[(1, 1)])
        for p in outproj_pair(1, 1, ats_pairs[(1, 1)]):
            p()
        if dbg2 is not None:
            for b in range(B):
                for pr in range(2):
                    nc.sync.dma_start(
                        dbg2[(b * 2 + pr) * 128 : (b * 2 + pr + 1) * 128, :],
                        ats_pairs[(b, pr)][:],
                    )

    orig_to_json = nc.to_json_bytes
    nc.to_json_bytes = lambda: _legalize_waits(orig_to_json())
    return nc


def _get_nc():
    if "nc" not in _CACHE:
        _CACHE["nc"] = _build()
    return _CACHE["nc"]


def _make_in_maps(inputs):
    q = np.asarray(inputs["q"], dtype=np.float32)
    v = np.asarray(inputs["v"], dtype=np.float32)
    k = np.asarray(inputs["k"], dtype=np.float32)
    w_query = np.asarray(inputs["w_query"], dtype=np.float32)
    b_query = np.asarray(inputs["b_query"], dtype=np.float32)
    w_value = np.asarray(inputs["w_value"], dtype=np.float32)
    b_value = np.asarray(inputs["b_value"], dtype=np.float32)
    w_key = np.asarray(inputs["w_key"], dtype=np.float32)
    b_key = np.asarray(inputs["b_key"], dtype=np.float32)
    w_projection = np.asarray(inputs["w_projection"], dtype=np.float32)
    b_projection = np.asarray(inputs["b_projection"], dtype=np.float32)

    scale = np.float32(1.0 / np.sqrt(H))

    def arrange_w(w):
        # [D or N*H, m] -> SBUF layout [128, (chunk, m)], contiguous rows
        m = w.shape[1]
        return np.ascontiguousarray(
            w.reshape(-1, 128, m).transpose(1, 0, 2).reshape(128, -1)
        ).astype(BF16)

    def arrange_x(xb):
        # [T, D] -> [D, T] -> SBUF layout [128, (tb, dc, t)], contiguous rows
        return np.ascontiguousarray(
            xb.T.reshape(NDC, 128, NTQB, XB)
            .transpose(1, 2, 0, 3)
            .reshape(128, NTQB * NDC * XB)
        ).astype(BF16)

    wp_s = arrange_w(
        np.ascontiguousarray(
            w_projection.transpose(0, 2, 1).reshape(N_HEADS * H, D)
        )
    )
    bp_s = np.ascontiguousarray(
        np.tile(b_projection.reshape(1, D), (128, 1))
    ).astype(np.float32)

    xT = {}
    for b in range(B):
        xT[b] = tuple(arrange_x(x[b]) for x in (q, k, v))

    in_maps = []
    for c in range(NCORES):
        hs = c * NLOC
        wq_s = arrange_w(w_query[:, hs : hs + NLOC, :].reshape(D, NW) * scale)
        wk_s = arrange_w(w_key[:, hs : hs + NLOC, :].reshape(D, NW))
        wv_s = arrange_w(w_value[:, hs : hs + NLOC, :].reshape(D, NW))
        bq_s = np.ascontiguousarray(
            (b_query[hs : hs + NLOC].reshape(NW) * scale).reshape(NW, 1)
        ).astype(np.float32)
        bk_s = np.ascontiguousarray(
            b_key[hs : hs + NLOC].reshape(NW, 1)
        ).astype(np.float32)
        bv_s = np.ascontiguousarray(
            b_value[hs : hs + NLOC].reshape(NW, 1)
        ).astype(np.float32)
        m = {
            "ident": np.eye(128, dtype=np.float32).astype(BF16),
            "identf": np.eye(128, dtype=np.float32),
            "wq": np.ascontiguousarray(wq_s),
            "wk": np.ascontiguousarray(wk_s),
            "wv": np.ascontiguousarray(wv_s),
            "wp": wp_s,
            "bq": bq_s,
            "bk": bk_s,
            "bv": bv_s,
            "bp": bp_s,
        }
        for b in range(B):
            m[f"qT{b}"], m[f"kT{b}"], m[f"vT{b}"] = xT[b]
        in_maps.append(m)
    return in_maps


def _assemble(results):
    out = np.empty((B, T, D), np.float32)
    for c in range(NCORES):
        res = results[c]["out"]  # [B*2*128, D]: rows (b, pair, i)
        for b in range(B):
            for pr in range(2):
                blk = res[(b * 2 + pr) * 128 : (b * 2 + pr + 1) * 128]
                for half in range(2):
                    j = 2 * pr + half
                    r0 = j * TQB + c * OWN
                    out[b, r0 : r0 + OWN, :] = blk[half * OWN : (half + 1) * OWN]
    return out


def run(inputs, trace=False, **kwargs):
    from concourse.bass_utils import run_bass_kernel_spmd

    nc = _get_nc()
    in_maps = _make_in_maps(inputs)
    res = run_bass_kernel_spmd(
        nc, in_maps, list(range(NCORES)), trace=trace, **kwargs
    )
    return _assemble(res.results), res


def kernel(**inputs) -> np.ndarray:
    out, _ = run(inputs, trace=False)
    return out


# revision 34
# speedup vs baseline: 1.0370x; 1.0370x over previous
"""Bass/Tile TRN2 kernel: 16-head MHA (B=2, T=2048, D=1024, H=64) on 8 NeuronCores.

Sharding: 8-way tensor parallel over heads — core c computes heads {2c, 2c+1}
for BOTH batches. Output ownership: within every (batch, 512-row tq block),
core c owns the 64 rows [c*64, (c+1)*64). After each block's attention one
small (128KB) AllToAll exchanges head-shards for row-shards, so every
collective except the last overlaps later attention blocks, and the output
projection for a PAIR of blocks (2x64 owned rows = 128 partitions) runs as
PE filler work.

Per-core pipeline (bf16 into the PE, fp32 PSUM accumulation):
  - ~200 tiny warm-up matmuls at t=0 trip the PE HAM clock gate to 2.4 GHz
    before the first projection.
  - DMA order: k0,q0 first so scores start ~15us in; k/v interleaved next at
    the rate the attention pipeline consumes them; wp before the first
    output-projection filler is needed.
  - QKV projections: 8x [128,128]x[128,512] accumulating matmuls per block.
  - Scores S^T[tk, tq] = K^T.T @ Q^T per head ([64,128] stationary, auto
    row-group packing); 1/sqrt(H) folded into Wq/bq on host.
  - exp on ScalarE straight out of PSUM in 1024-wide ACTIVATEs; the two
    heads' score PSUM buffers alternate so ACT never waits on score matmuls.
    Attention is emitted as ONE flat software pipeline over all 8 (b, tq)
    blocks: the next block's scores/exps are emitted before the previous
    block's normalize, so ACT stays saturated across block boundaries.
  - PV matmul with a ones-augmented V (stationary col 64 = ones) so row 64 of
    the PV accumulator is the softmax denominator for free.
  - Normalize: pv evacuated by DVE (frees the PSUM bank), denominator row
    PE-transposed to [128,4] for a cheap DVE reciprocal, broadcast back via a
    1-row outer-product matmul, multiply, stage into the block's AllToAll
    buffer as [8 dest cores x 128 headdim, 64 tq].
  - PE idle slots inside the ACT-bound phase are filled with the next batch's
    projections and completed pairs' output projections.
Host does layout-only prep (transpose, bf16 cast, weight slicing) and
scatters the 8 cores' per-block 64-row output slices.
"""

import sys
from contextlib import ExitStack

import numpy as np

sys.path.insert(0, "/opt/trn_rl_repo")

import ml_dtypes  # noqa: E402

BF16 = ml_dtypes.bfloat16

B, T, D = 2, 2048, 1024
N_HEADS, H = 16, 64
NCORES = 8
GROUPS = [[0, 1, 2, 3, 4, 5, 6, 7]]
NLOC = 2            # heads per core
TQB = 512           # attention tq block
NTQB = T // TQB     # 4
TKC = 128           # tk chunk
NTKC = T // TKC     # 16
RG = 2              # tk chunks per exp group
NG = NTKC // RG     # 8 groups per block
DC = 128            # d chunk
NDC = D // DC       # 8
XB = 512            # x-load column block
OWN = 64            # tq rows per core per block
VA = 128            # V_aug stationary width: [V(64) | ones(1) | junk(63)]
NW = NLOC * H       # 128 projection width per core
WARMUP = 200        # HAM pre-warm matmul count (0 = off)
DEBUG_AN = False    # dump normalized attention tiles to a debug output

_CACHE = {}


def _legalize_waits(bir_bytes):
    """This toolchain's walrus accepts at most ONE semaphore wait per
    instruction ("Too many sync wait commands"). Tile's sem assignment emits
    several. Hoist all but one wait of each instruction onto same-engine NoOps
    inserted immediately before it (engines execute their stream in order, so
    waiting earlier on the same engine is equivalent)."""
    import json

    j = json.loads(bir_bytes)
    ctr = 0
    for fn in j["functions"]:
        for blk in fn["blocks"]:
            out = []
            for ins in blk["instructions"]:
                si = ins.get("sync_info")
                waits = (si or {}).get("on_wait") or []
                if len(waits) > 1:
                    for w in waits[:-1]:
                        ctr += 1
                        out.append(
                            {
                                "engine": ins["engine"],
                                "ins": [],
                                "outs": [],
                                "name": f"waitfix-{ctr}",
                                "opcode": "NoOp",
                                "sync_info": {"on_wait": [w], "on_update": []},
                            }
                        )
                    si["on_wait"] = [waits[-1]]
                out.append(ins)
            blk["instructions"] = out
    return json.dumps(j).encode()


def _build():
    import concourse.bass as bass
    import concourse.mybir as mybir
    import concourse.tile as tile

    f32 = mybir.dt.float32
    bf16 = mybir.dt.bfloat16
    AF = mybir.ActivationFunctionType
    ALU = mybir.AluOpType

    nc = bass.Bass(
        "TRN2", target_bir_lowering=False, debug=False, num_devices=NCORES
    )

    # activations/weights arrive pre-arranged on host into the exact SBUF
    # layout ([partition, ...] contiguous) so every load is a 1:1 DMA with
    # 2KB+ lines and 128 descriptors.
    qT = [nc.dram_tensor(f"qT{b}", [128, NTQB * NDC * XB], bf16, kind="ExternalInput") for b in range(B)]
    kT = [nc.dram_tensor(f"kT{b}", [128, NTQB * NDC * XB], bf16, kind="ExternalInput") for b in range(B)]
    vT = [nc.dram_tensor(f"vT{b}", [128, NTQB * NDC * XB], bf16, kind="ExternalInput") for b in range(B)]
    wq = nc.dram_tensor("wq", [128, NDC * NW], bf16, kind="ExternalInput")
    wk = nc.dram_tensor("wk", [128, NDC * NW], bf16, kind="ExternalInput")
    wv = nc.dram_tensor("wv", [128, NDC * NW], bf16, kind="ExternalInput")
    wp = nc.dram_tensor("wp", [128, (N_HEADS * H // 128) * D], bf16, kind="ExternalInput")
    bq = nc.dram_tensor("bq", [128, 1], f32, kind="ExternalInput")
    bk = nc.dram_tensor("bk", [128, 1], f32, kind="ExternalInput")
    bv = nc.dram_tensor("bv", [128, 1], f32, kind="ExternalInput")
    bp = nc.dram_tensor("bp", [128, D], f32, kind="ExternalInput")
    ident = nc.dram_tensor("ident", [128, 128], bf16, kind="ExternalInput")
    identf = nc.dram_tensor("identf", [128, 128], f32, kind="ExternalInput")
    # rows: (b, pair, i) with i in [0,128): j = 2*pair + i//64, own-row i%64
    out = nc.dram_tensor("out", [B * 2 * 128, D], f32, kind="ExternalOutput")
    dbg = (
        nc.dram_tensor("dbg", [B * NTQB * NLOC * H, TQB], bf16, kind="ExternalOutput")
        if DEBUG_AN
        else None
    )
    dbg2 = (
        nc.dram_tensor("dbg2", [4 * 128, NCORES * 128], bf16, kind="ExternalOutput")
        if DEBUG_AN
        else None
    )

    with tile.TileContext(nc) as tc, ExitStack() as ctx:
        p_const = ctx.enter_context(tc.tile_pool(name="const", bufs=1))
        p_x = ctx.enter_context(tc.tile_pool(name="x", bufs=1))
        p_qk = ctx.enter_context(tc.tile_pool(name="qk", bufs=2))
        p_va = ctx.enter_context(tc.tile_pool(name="va", bufs=2))
        p_pt = ctx.enter_context(tc.tile_pool(name="pt", bufs=3))
        p_a = ctx.enter_context(tc.tile_pool(name="a", bufs=2))
        p_o = ctx.enter_context(tc.tile_pool(name="o", bufs=2))
        p_dram = ctx.enter_context(tc.tile_pool(name="dram", bufs=1, space="DRAM"))

        ps_ss = ctx.enter_context(tc.tile_pool(name="ps_ss", bufs=1, space="PSUM"))
        ps_pv = ctx.enter_context(tc.tile_pool(name="ps_pv", bufs=1, space="PSUM"))
        ps_mm = ctx.enter_context(tc.tile_pool(name="ps_mm", bufs=2, space="PSUM"))

        # ---- constant tiles -------------------------------------------------
        wq_sb = p_const.tile([128, NDC * NW], bf16)
        wk_sb = p_const.tile([128, NDC * NW], bf16)
        wv_sb = p_const.tile([128, NDC * NW], bf16)
        wp_sb = p_const.tile([128, (N_HEADS * H // 128) * D], bf16)
        bq_sb = p_const.tile([128, 1], f32)
        bk_sb = p_const.tile([128, 1], f32)
        bv_sb = p_const.tile([128, 1], f32)
        bp_sb = p_const.tile([128, D], f32)
        id_sb = p_const.tile([128, 128], bf16)
        idf_sb = p_const.tile([128, 128], f32)

        # warm the ACT exp table while everything else is still loading
        warm = p_const.tile([1, 8], bf16)
        nc.vector.memset(warm[:], 0.0)
        nc.scalar.activation(warm[:], warm[:], AF.Exp)

        # ones tiles: f32 (K=1 transpose "identity") and bf16 (broadcast
        # outer-product column, 1 col/cycle instead of fp32's 1/2)
        onesf = p_const.tile([128, H], f32)
        nc.vector.memset(onesf[:], 1.0)
        onesb = p_const.tile([128, H], bf16)
        nc.vector.memset(onesb[:], 1.0)

        # ---- HAM pre-warm: keep the PE busy from t~0 so the clock gate is
        # released (1.2 -> 2.4 GHz) before the first real projection matmul.
        if WARMUP:
            wtile = p_const.tile([128, 64], bf16)
            nc.vector.memset(wtile[:], 0.0)
            wps = ps_mm.tile([64, 64], f32, name="wps", tag="mm")
            for _ in range(WARMUP):
                nc.tensor.matmul(wps[:], lhsT=wtile[:, 0:64], rhs=wtile[:], start=True, stop=True)

        BLOCKS = [(b, j) for b in range(B) for j in range(NTQB)]
        # Pair-level exchange buffers: [8 src cores x 128 headdim,
        # (half, own-64)] per (batch, pair-of-blocks). 4 collectives of 256KB
        # each — the mesh AllToAll has ~15-30us fixed cost per op, so
        # per-block (8x128KB) exchanges serialize into the critical path.
        # NOTE: untagged tiles in one pool share a single slot ring — every
        # DRAM tile needs its own tag or all collectives alias one buffer.
        PAIRS = [(b, pr) for b in range(B) for pr in range(2)]
        a2a_in = {
            pp: p_dram.tile(
                [NCORES * NW, 2 * OWN], bf16,
                name=f"a2ai{pp[0]}{pp[1]}", tag=f"a2ai{pp[0]}{pp[1]}",
            )
            for pp in PAIRS
        }
        a2a_out = {
            pp: p_dram.tile(
                [NCORES * NW, 2 * OWN], bf16,
                name=f"a2ao{pp[0]}{pp[1]}", tag=f"a2ao{pp[0]}{pp[1]}",
            )
            for pp in PAIRS
        }

        # ---- x loads: per (tensor, tb) contiguous 8KB-line chunks -----------
        CW = NDC * XB  # 4096 cols per tb chunk
        exts = {"v": vT, "k": kT, "q": qT}
        xs = {0: {}, 1: {}}

        def load_x_one(b, tname, tb):
            # q tiles ride a 2-buffer ring (q2/q3 arrive ~25us after Q0/Q1
            # projected — no WAR stall); k/v keep per-tb buffers so the
            # in-order load queue never waits on a projection mid-stream.
            tg = f"x{tname}{tb % 2}" if tname == "q" else f"x{tname}{tb}"
            t_ = p_x.tile([128, CW], bf16, name=f"x{tname}{b}{tb}", tag=tg)
            nc.sync.dma_start(t_[:], exts[tname][b][:, tb * CW : (tb + 1) * CW])
            xs[b][(tname, tb)] = t_

        def load_startup():
            """Loads in consumption order. b=0: k0,q0 first (scores start as
            soon as K0/Q0 project), then k/v interleaved at the rate block
            (0,0) consumes chunks, then q1-3 (needed from block (0,1)).
            b=1 x-loads reuse b=0's buffers (same tag -> WAR on b0 proj).
            wp (2MB) before b1 q-loads: first outproj filler fires ~85us in."""
            nc.sync.dma_start(id_sb[:], ident[:])
            nc.sync.dma_start(wk_sb[:], wk[:])
            nc.sync.dma_start(bk_sb[:], bk[:])
            load_x_one(0, "k", 0)
            nc.sync.dma_start(wq_sb[:], wq[:])
            nc.sync.dma_start(bq_sb[:], bq[:])
            load_x_one(0, "q", 0)
            load_x_one(0, "k", 1)
            nc.sync.dma_start(wv_sb[:], wv[:])
            nc.sync.dma_start(bv_sb[:], bv[:])
            load_x_one(0, "v", 0)
            load_x_one(0, "v", 1)
            load_x_one(0, "k", 2)
            load_x_one(0, "v", 2)
            load_x_one(0, "k", 3)
            load_x_one(0, "q", 1)
            load_x_one(0, "v", 3)
            load_x_one(0, "q", 2)
            load_x_one(0, "q", 3)
            nc.sync.dma_start(idf_sb[:], identf[:])
            nc.sync.dma_start(bp_sb[:], bp[:])
            for tb in range(NTQB):
                load_x_one(1, "v", tb)
                load_x_one(1, "k", tb)
            nc.sync.dma_start(wp_sb[:], wp[:])
            for tb in range(NTQB):
                load_x_one(1, "q", tb)

        # ---- projection pieces (emitted inline or as PE fillers) ------------
        def proj_block(w_sb, b_sb, xt, dst, dcol, b, tag):
            ps = ps_mm.tile([128, XB], f32, name=f"mm{tag}{b}{dcol}", tag="mm")
            for dc in range(NDC):
                nc.tensor.matmul(
                    ps[:],
                    lhsT=w_sb[:, dc * NW : (dc + 1) * NW],
                    rhs=xt[:, dc * XB : (dc + 1) * XB],
                    start=(dc == 0),
                    stop=(dc == NDC - 1),
                )
            nc.vector.tensor_scalar(
                dst[:, dcol * XB : (dcol + 1) * XB], ps[:], b_sb[:, 0:1], None,
                ALU.add,
            )

        def va_piece(b, vt, va, i0, n):
            """Transpose tk-chunks [i0, i0+n) of vt into va."""
            for i in range(i0, i0 + n):
                pst = ps_mm.tile([128, 128], bf16, name=f"pst{b}{i}", tag="mm")
                nc.tensor.transpose(pst[:], vt[:, i * TKC : (i + 1) * TKC], id_sb[:])
                dst = va[:, i * NLOC * VA : (i + 1) * NLOC * VA].rearrange(
                    "p (h x) -> p h x", x=VA
                )[:, :, 0:H]
                nc.vector.tensor_copy(dst, pst[:].rearrange("p (h x) -> p h x", x=H))

        def make_proj(b):
            """Returns (qt, kt, va, pieces): pieces in pipeline-consumption
            order (K first, then V/va interleaved, Q last)."""
            vt = p_va.tile([128, T], bf16, name=f"vt{b}", tag="vt")
            va = p_va.tile([128, NTKC * NLOC * VA], bf16, name=f"va{b}", tag="va")
            qt = p_qk.tile([128, T], bf16, name=f"qt{b}", tag="qt")
            kt = p_qk.tile([128, T], bf16, name=f"kt{b}", tag="kt")
            nc.vector.memset(
                va[:].rearrange("p (i h x) -> p i h x", h=NLOC, x=VA)[
                    :, :, :, H : H + 1
                ],
                1.0,
            )
            mk = lambda w, bb, tn, dst, tb, tg: (
                lambda: proj_block(w, bb, xs[b][(tn, tb)], dst, tb, b, tg)
            )
            # consumption order matches the DMA arrival order (~1 chunk per
            # exp group): V tb / va / K tb interleaved, Q blocks last.
            pieces = [
                mk(wk_sb, bk_sb, "k", kt, 0, "k"),
                mk(wq_sb, bq_sb, "q", qt, 0, "q"),
                mk(wv_sb, bv_sb, "v", vt, 0, "v"),
                lambda: va_piece(b, vt, va, 0, 4),
                mk(wk_sb, bk_sb, "k", kt, 1, "k"),
                mk(wv_sb, bv_sb, "v", vt, 1, "v"),
                lambda: va_piece(b, vt, va, 4, 4),
                mk(wk_sb, bk_sb, "k", kt, 2, "k"),
                mk(wv_sb, bv_sb, "v", vt, 2, "v"),
                lambda: va_piece(b, vt, va, 8, 4),
                mk(wk_sb, bk_sb, "k", kt, 3, "k"),
                mk(wv_sb, bv_sb, "v", vt, 3, "v"),
                lambda: va_piece(b, vt, va, 12, 4),
                mk(wq_sb, bq_sb, "q", qt, 1, "q"),
                mk(wq_sb, bq_sb, "q", qt, 2, "q"),
                mk(wq_sb, bq_sb, "q", qt, 3, "q"),
            ]
            return qt, kt, va, pieces

        fillers = []

        def run_filler(n=1):
            for _ in range(n):
                if fillers:
                    fillers.pop(0)()

        # ---- output projection for a PAIR of blocks -------------------------
        # ats_pair[:, s*128 + half*64 : +64] holds src-core s's heads for the
        # half-th block of the pair; one [128,512] matmul group per D-half.
        def outproj_pair(b, pr, ats_pair):
            def op(dh):
                ps = ps_mm.tile([128, 512], f32, name=f"po{b}{pr}{dh}", tag="mm")
                for s in range(NCORES):
                    nc.tensor.matmul(
                        ps[:],
                        lhsT=ats_pair[:, s * 128 : (s + 1) * 128],
                        rhs=wp_sb[:, s * D + dh * 512 : s * D + (dh + 1) * 512],
                        start=(s == 0),
                        stop=(s == NCORES - 1),
                    )
                o_sb = p_o.tile([128, 512], f32, name=f"o{b}{pr}{dh}", tag="o")
                nc.vector.tensor_tensor(
                    o_sb[:], ps[:], bp_sb[:, dh * 512 : (dh + 1) * 512], ALU.add
                )
                nc.sync.dma_start(
                    out[
                        (b * 2 + pr) * 128 : (b * 2 + pr + 1) * 128,
                        dh * 512 : (dh + 1) * 512,
                    ],
                    o_sb[:],
                )
            return [lambda dh=dh: op(dh) for dh in range(2)]

        def ats_load(b, pr, ats_pair):
            for s in range(NCORES):
                nc.sync.dma_start(
                    ats_pair[:, s * 128 : (s + 1) * 128],
                    a2a_out[(b, pr)][s * 128 : (s + 1) * 128, :],
                )

        # ---- attention pieces ----------------------------------------------
        st = {}  # live per-block state: pv tiles, pss, pt, a_sb

        def scores(bl, g, qt, kt):
            # head-major: each head's chunk pair releases together (gated by
            # its exp) and the two MMs pipeline fill-under-drain.
            b, j = bl
            for hd in range(NLOC):
                key = (bl, g % 2, hd)
                st[("ss",) + key] = ps_ss.tile(
                    [128, RG * TQB], f32,
                    name=f"pss{b}{j}{g}{hd}", tag=f"ss{hd}",
                )
                for r in range(g * RG, (g + 1) * RG):
                    nc.tensor.matmul(
                        st[("ss",) + key][:, (r % RG) * TQB : (r % RG + 1) * TQB],
                        lhsT=kt[hd * H : (hd + 1) * H, r * TKC : (r + 1) * TKC],
                        rhs=qt[hd * H : (hd + 1) * H, j * TQB : (j + 1) * TQB],
                        start=True,
                        stop=True,
                    )

        def exps(bl, g):
            b, j = bl
            for hd in range(NLOC):
                pt_t = p_pt.tile(
                    [128, RG * TQB], bf16,
                    name=f"pt{b}{j}{g}{hd}", tag=f"pt{hd}",
                )
                st[("pt", bl, g % 2, hd)] = pt_t
                nc.scalar.activation(
                    pt_t[:], st[("ss", bl, g % 2, hd)][:], AF.Exp
                )

        def pvs(bl, g, va):
            b, j = bl
            if g == 0:
                st[("pv", bl)] = [
                    ps_pv.tile([VA, TQB], f32, name=f"pv{b}{j}{hd}", tag=f"pv{hd}")
                    for hd in range(NLOC)
                ]
            pv = st[("pv", bl)]
            for hd in range(NLOC):
                for r in range(g * RG, (g + 1) * RG):
                    col0 = (r * NLOC + hd) * VA
                    nc.tensor.matmul(
                        pv[hd][:],
                        lhsT=va[:, col0 : col0 + VA],
                        rhs=st[("pt", bl, g % 2, hd)][:, (r % RG) * TQB : (r % RG + 1) * TQB],
                        start=(g == 0 and r == g * RG),
                        stop=(g == NG - 1 and r == (g + 1) * RG - 1),
                    )

        def norm_a(bl):
            """Evacuate pv -> SBUF (frees the pv PSUM banks for the next
            block's accumulation). 96 partitions so the denominator row (64)
            sits in a 32-aligned window for the DVE block-transpose."""
            b, j = bl
            pv = st.pop(("pv", bl))
            for hd in range(NLOC):
                a_sb = p_a.tile(
                    [96, TQB], f32, name=f"as{b}{j}{hd}", tag=f"as{hd}"
                )
                nc.vector.tensor_copy(a_sb[0 : H + 1, :], pv[hd][0 : H + 1, :])
                st[("as", bl, hd)] = a_sb

        def norm_b(bl, hd):
            """Reciprocal of the denominator row, all on DVE: 32x32
            block-transpose puts den[32j+r] at [r, 32j], a strided reciprocal
            hits only those lanes, a second block-transpose puts 1/den back
            as row 0; an fp32 outer-product matmul broadcasts it to H rows
            for the multiply. Then stage into the pair's AllToAll input."""
            b, j = bl
            a_sb = st.pop(("as", bl, hd))
            tr = p_a.tile([32, TQB], f32, name=f"tr{b}{j}{hd}", tag=f"tr{hd}", bufs=1)
            nc.vector.transpose(tr[:], a_sb[64:96, :])
            rc = p_a.tile([32, TQB], f32, name=f"rc{b}{j}{hd}", tag=f"rc{hd}", bufs=1)
            nc.vector.reciprocal(
                rc[:].rearrange("p (j c) -> p j c", c=32)[:, :, 0:1],
                tr[:].rearrange("p (j c) -> p j c", c=32)[:, :, 0:1],
            )
            rw = p_a.tile([32, TQB], f32, name=f"rw{b}{j}{hd}", tag=f"rw{hd}", bufs=1)
            nc.vector.transpose(rw[:], rc[:])
            rep_ps = ps_mm.tile([H, TQB], f32, name=f"rp{b}{j}{hd}", tag="mm")
            nc.tensor.matmul(
                rep_ps[:], lhsT=onesf[0:1, 0:H], rhs=rw[0:1, :], start=True, stop=True
            )
            an = p_a.tile([H, TQB], bf16, name=f"an{b}{j}{hd}", tag=f"an{hd}")
            nc.vector.tensor_tensor(an[:], a_sb[0:H, :], rep_ps[:], ALU.mult)
            if dbg is not None:
                r0 = ((b * NTQB + j) * NLOC + hd) * H
                nc.sync.dma_start(dbg[r0 : r0 + H, :], an[:])
            # stage into the pair buffer: rows c*128 + hd*64 + h, cols
            # (j%2)*64 + own-64 of tq. Iteration order (h, c, t) on both
            # sides; SBUF AP keeps the partition dim (h) first.
            half = j % 2
            dst = a2a_in[(b, j // 2)].rearrange("(c s) t -> s c t", s=NW)[
                hd * H : (hd + 1) * H, :, half * OWN : (half + 1) * OWN
            ]
            src = an[:].rearrange("h (c t) -> h c t", t=OWN)
            # SP queue: behind the x-loads (first ~75us), which is fine — the
            # collective chain is trigger-paced after that. (scalar/gpsimd
            # DMA queues corrupt this strided transfer: NaN / wrong layout.)
            nc.sync.dma_start(dst, src)

        def collective(pp):
            nc.gpsimd.collective_compute(
                "AllToAll",
                mybir.AluOpType.bypass,
                replica_groups=GROUPS,
                ins=[a2a_in[pp].opt()],
                outs=[a2a_out[pp].opt()],
            )

        # ===== main schedule =================================================
        load_startup()
        qt0, kt0, va0, pieces0 = make_proj(0)
        # inline: K0, Q0 only -- scores start as soon as they project.
        pieces0[0]()
        pieces0[1]()
        # b0 filler order = consumption order (V0/K1/va03 lead; Q1/Q2 hoisted
        # so each lands >=1 group before its block's first scores).
        fillers.extend(
            [pieces0[i] for i in (2, 4, 3, 7, 5, 6, 8, 10, 9, 13, 11, 12, 14, 15)]
        )

        qt1, kt1, va1, p1 = make_proj(1)
        # b1 consumption order matching its v/k-interleaved DMA arrival
        fillers.extend(
            [p1[i] for i in (2, 0, 3, 5, 4, 6, 8, 7, 9, 11, 10, 12, 1, 13, 14, 15)]
        )

        # EMISSION ORDER IS DATAFLOW: a filler emitted after its consumer's
        # emission silently reads stale data. Pops per (bi, g), hand-paced to
        # DMA arrival while keeping >=1 group of margin before each consumer.
        # bi=0 list: [V0, K1, va03 | - | K2, V1, va47 | V2 | K3, va811 | Q1 |
        #             V3, va1215, Q2 | -]
        POPS = {
            0: [3, 0, 3, 1, 2, 1, 3, 0],
            1: [0, 0, 0, 2, 1, 2, 1, 1],
            2: [0, 0, 0, 1, 1, 1, 1, 1],
            3: [0, 0, 0, 1, 1, 1, 1, 1],
        }
        DEF_POPS = [0, 0, 0, 0, 2, 2, 2, 2]

        qkv = {0: (qt0, kt0, va0), 1: (qt1, kt1, va1)}
        ats_pairs = {}
        for b in range(B):
            for pr in range(2):
                ats_pairs[(b, pr)] = p_a.tile(
                    [128, NCORES * 128], bf16, name=f"ats{b}{pr}", tag=f"ats{pr}"
                )

        NB = len(BLOCKS)
        scores(BLOCKS[0], 0, qt0, kt0)
        exps(BLOCKS[0], 0)
        for bi, bl in enumerate(BLOCKS):
            b, j = bl
            qt, kt, va = qkv[b]
            for g in range(NG):
                # one-group lookahead (crosses block boundaries)
                if g + 1 < NG:
                    scores(bl, g + 1, qt, kt)
                    exps(bl, g + 1)
                elif bi + 1 < NB:
                    nbl = BLOCKS[bi + 1]
                    nqt, nkt, _ = qkv[nbl[0]]
                    scores(nbl, 0, nqt, nkt)
                    exps(nbl, 0)
                if bi > 0:
                    pbl = BLOCKS[bi - 1]
                    if g == 0:
                        norm_b(pbl, 0)
                    elif g == 1:
                        norm_b(pbl, 1)
                    elif g == 2 and pbl[1] % 2 == 1:
                        collective((pbl[0], pbl[1] // 2))
                    elif g == 3 and bi in (4, 6):
                        # ats for the pair exchanged TWO blocks ago (its
                        # collective is long done -> no SP-queue stall)
                        pb, pj = BLOCKS[bi - 3]
                        pr = pj // 2
                        ats_load(pb, pr, ats_pairs[(pb, pr)])
                        fillers.extend(outproj_pair(pb, pr, ats_pairs[(pb, pr)]))
                run_filler(POPS.get(bi, DEF_POPS)[g])
                pvs(bl, g, va)
                if g == NG - 1:
                    norm_a(bl)

        # ---- tail: last block's norm + pair collectives + outprojs ---------
        last = BLOCKS[-1]
        norm_b(last, 0)
        norm_b(last, 1)
        collective((1, 1))
        # pair (1,0)'s collective was triggered two blocks ago; its outproj
        # runs while pair (1,1)'s collective is in flight.
        ats_load(1, 0, ats_pairs[(1, 0)])
        for p in outproj_pair(1, 0, ats_pairs[(1, 0)]):
            p()
        run_filler(len(fillers))
        ats_load(1, 1, ats_pairs[(1, 1)])
        for p in outproj_pair(1, 1, ats_pairs[(1, 1)]):
            p()
        if dbg2 is not None:
            for b in range(B):
                for pr in range(2):
                    nc.sync.dma_start(
                        dbg2[(b * 2 + pr) * 128 : (b * 2 + pr + 1) * 128, :],
                        ats_pairs[(b, pr)][:],
                    )

    orig_to_json = nc.to_json_bytes
    nc.to_json_bytes = lambda: _legalize_waits(orig_to_json())
    return nc


def _get_nc():
    if "nc" not in _CACHE:
        _CACHE["nc"] = _build()
    return _CACHE["nc"]


def _make_in_maps(inputs):
    q = np.asarray(inputs["q"], dtype=np.float32)
    v = np.asarray(inputs["v"], dtype=np.float32)
    k = np.asarray(inputs["k"], dtype=np.float32)
    w_query = np.asarray(inputs["w_query"], dtype=np.float32)
    b_query = np.asarray(inputs["b_query"], dtype=np.float32)
    w_value = np.asarray(inputs["w_value"], dtype=np.float32)
    b_value = np.asarray(inputs["b_value"], dtype=np.float32)
    w_key = np.asarray(inputs["w_key"], dtype=np.float32)
    b_key = np.asarray(inputs["b_key"], dtype=np.float32)
    w_projection = np.asarray(inputs["w_projection"], dtype=np.float32)
    b_projection = np.asarray(inputs["b_projection"], dtype=np.float32)

    scale = np.float32(1.0 / np.sqrt(H))

    def arrange_w(w):
        # [D or N*H, m] -> SBUF layout [128, (chunk, m)], contiguous rows
        m = w.shape[1]
        return np.ascontiguousarray(
            w.reshape(-1, 128, m).transpose(1, 0, 2).reshape(128, -1)
        ).astype(BF16)

    def arrange_x(xb):
        # [T, D] -> [D, T] -> SBUF layout [128, (tb, dc, t)], contiguous rows
        return np.ascontiguousarray(
            xb.T.reshape(NDC, 128, NTQB, XB)
            .transpose(1, 2, 0, 3)
            .reshape(128, NTQB * NDC * XB)
        ).astype(BF16)

    wp_s = arrange_w(
        np.ascontiguousarray(
            w_projection.transpose(0, 2, 1).reshape(N_HEADS * H, D)
        )
    )
    bp_s = np.ascontiguousarray(
        np.tile(b_projection.reshape(1, D), (128, 1))
    ).astype(np.float32)

    xT = {}
    for b in range(B):
        xT[b] = tuple(arrange_x(x[b]) for x in (q, k, v))

    in_maps = []
    for c in range(NCORES):
        hs = c * NLOC
        wq_s = arrange_w(w_query[:, hs : hs + NLOC, :].reshape(D, NW) * scale)
        wk_s = arrange_w(w_key[:, hs : hs + NLOC, :].reshape(D, NW))
        wv_s = arrange_w(w_value[:, hs : hs + NLOC, :].reshape(D, NW))
        bq_s = np.ascontiguousarray(
            (b_query[hs : hs + NLOC].reshape(NW) * scale).reshape(NW, 1)
        ).astype(np.float32)
        bk_s = np.ascontiguousarray(
            b_key[hs : hs + NLOC].reshape(NW, 1)
        ).astype(np.float32)
        bv_s = np.ascontiguousarray(
            b_value[hs : hs + NLOC].reshape(NW, 1)
        ).astype(np.float32)
        m = {
            "ident": np.eye(128, dtype=np.float32).astype(BF16),
            "identf": np.eye(128, dtype=np.float32),
            "wq": np.ascontiguousarray(wq_s),
            "wk": np.ascontiguousarray(wk_s),
            "wv": np.ascontiguousarray(wv_s),
            "wp": wp_s,
            "bq": bq_s,
            "bk": bk_s,
            "bv": bv_s,
            "bp": bp_s,
        }
        for b in range(B):
            m[f"qT{b}"], m[f"kT{b}"], m[f"vT{b}"] = xT[b]
        in_maps.append(m)
    return in_maps


def _assemble(results):
    out = np.empty((B, T, D), np.float32)
    for c in range(NCORES):
        res = results[c]["out"]  # [B*2*128, D]: rows (b, pair, i)
        for b in range(B):
            for pr in range(2):
                blk = res[(b * 2 + pr) * 128 : (b * 2 + pr + 1) * 128]
                for half in range(2):
                    j = 2 * pr + half
                    r0 = j * TQB + c * OWN
                    out[b, r0 : r0 + OWN, :] = blk[half * OWN : (half + 1) * OWN]
    return out


def run(inputs, trace=False, **kwargs):
    from concourse.bass_utils import run_bass_kernel_spmd

    nc = _get_nc()
    in_maps = _make_in_maps(inputs)
    res = run_bass_kernel_spmd(
        nc, in_maps, list(range(NCORES)), trace=trace, **kwargs
    )
    return _assemble(res.results), res


def kernel(**inputs) -> np.ndarray:
    out, _ = run(inputs, trace=False)
    return out


# revision 36
# speedup vs baseline: 1.0494x; 1.0120x over previous
"""Bass/Tile TRN2 kernel: 16-head MHA (B=2, T=2048, D=1024, H=64) on 8 NeuronCores.

Sharding: 8-way tensor parallel over heads — core c computes heads {2c, 2c+1}
for BOTH batches. Output ownership: within every (batch, 512-row tq block),
core c owns the 64 rows [c*64, (c+1)*64). After each block's attention one
small (128KB) AllToAll exchanges head-shards for row-shards, so every
collective except the last overlaps later attention blocks, and the output
projection for a PAIR of blocks (2x64 owned rows = 128 partitions) runs as
PE filler work.

Per-core pipeline (bf16 into the PE, fp32 PSUM accumulation):
  - ~200 tiny warm-up matmuls at t=0 trip the PE HAM clock gate to 2.4 GHz
    before the first projection.
  - DMA order: k0,q0 first so scores start ~15us in; k/v interleaved next at
    the rate the attention pipeline consumes them; wp before the first
    output-projection filler is needed.
  - QKV projections: 8x [128,128]x[128,512] accumulating matmuls per block.
  - Scores S^T[tk, tq] = K^T.T @ Q^T per head ([64,128] stationary, auto
    row-group packing); 1/sqrt(H) folded into Wq/bq on host.
  - exp on ScalarE straight out of PSUM in 1024-wide ACTIVATEs; the two
    heads' score PSUM buffers alternate so ACT never waits on score matmuls.
    Attention is emitted as ONE flat software pipeline over all 8 (b, tq)
    blocks: the next block's scores/exps are emitted before the previous
    block's normalize, so ACT stays saturated across block boundaries.
  - PV matmul with a ones-augmented V (stationary col 64 = ones) so row 64 of
    the PV accumulator is the softmax denominator for free.
  - Normalize: pv evacuated by DVE (frees the PSUM bank), denominator row
    PE-transposed to [128,4] for a cheap DVE reciprocal, broadcast back via a
    1-row outer-product matmul, multiply, stage into the block's AllToAll
    buffer as [8 dest cores x 128 headdim, 64 tq].
  - PE idle slots inside the ACT-bound phase are filled with the next batch's
    projections and completed pairs' output projections.
Host does layout-only prep (transpose, bf16 cast, weight slicing) and
scatters the 8 cores' per-block 64-row output slices.
"""

import sys
from contextlib import ExitStack

import numpy as np

sys.path.insert(0, "/opt/trn_rl_repo")

import ml_dtypes  # noqa: E402

BF16 = ml_dtypes.bfloat16

B, T, D = 2, 2048, 1024
N_HEADS, H = 16, 64
NCORES = 8
GROUPS = [[0, 1, 2, 3, 4, 5, 6, 7]]
NLOC = 2            # heads per core
TQB = 512           # attention tq block
NTQB = T // TQB     # 4
TKC = 128           # tk chunk
NTKC = T // TKC     # 16
RG = 2              # tk chunks per exp group
NG = NTKC // RG     # 8 groups per block
DC = 128            # d chunk
NDC = D // DC       # 8
XB = 512            # x-load column block
OWN = 64            # tq rows per core per block
VA = 128            # V_aug stationary width: [V(64) | ones(1) | junk(63)]
NW = NLOC * H       # 128 projection width per core
WARMUP = 200        # HAM pre-warm matmul count (0 = off)
DEBUG_AN = False    # dump normalized attention tiles to a debug output

_CACHE = {}


def _legalize_waits(bir_bytes):
    """This toolchain's walrus accepts at most ONE semaphore wait per
    instruction ("Too many sync wait commands"). Tile's sem assignment emits
    several. Hoist all but one wait of each instruction onto same-engine NoOps
    inserted immediately before it (engines execute their stream in order, so
    waiting earlier on the same engine is equivalent)."""
    import json

    j = json.loads(bir_bytes)
    ctr = 0
    for fn in j["functions"]:
        for blk in fn["blocks"]:
            out = []
            for ins in blk["instructions"]:
                si = ins.get("sync_info")
                waits = (si or {}).get("on_wait") or []
                if len(waits) > 1:
                    for w in waits[:-1]:
                        ctr += 1
                        out.append(
                            {
                                "engine": ins["engine"],
                                "ins": [],
                                "outs": [],
                                "name": f"waitfix-{ctr}",
                                "opcode": "NoOp",
                                "sync_info": {"on_wait": [w], "on_update": []},
                            }
                        )
                    si["on_wait"] = [waits[-1]]
                out.append(ins)
            blk["instructions"] = out
    return json.dumps(j).encode()


def _build():
    import concourse.bass as bass
    import concourse.mybir as mybir
    import concourse.tile as tile

    f32 = mybir.dt.float32
    bf16 = mybir.dt.bfloat16
    AF = mybir.ActivationFunctionType
    ALU = mybir.AluOpType

    nc = bass.Bass(
        "TRN2", target_bir_lowering=False, debug=False, num_devices=NCORES
    )

    # activations/weights arrive pre-arranged on host into the exact SBUF
    # layout ([partition, ...] contiguous) so every load is a 1:1 DMA with
    # 2KB+ lines and 128 descriptors.
    qT = [nc.dram_tensor(f"qT{b}", [128, NTQB * NDC * XB], bf16, kind="ExternalInput") for b in range(B)]
    kT = [nc.dram_tensor(f"kT{b}", [128, NTQB * NDC * XB], bf16, kind="ExternalInput") for b in range(B)]
    vT = [nc.dram_tensor(f"vT{b}", [128, NTQB * NDC * XB], bf16, kind="ExternalInput") for b in range(B)]
    wq = nc.dram_tensor("wq", [128, NDC * NW], bf16, kind="ExternalInput")
    wk = nc.dram_tensor("wk", [128, NDC * NW], bf16, kind="ExternalInput")
    wv = nc.dram_tensor("wv", [128, NDC * NW], bf16, kind="ExternalInput")
    wp = nc.dram_tensor("wp", [128, (N_HEADS * H // 128) * D], bf16, kind="ExternalInput")
    bq = nc.dram_tensor("bq", [128, 1], f32, kind="ExternalInput")
    bk = nc.dram_tensor("bk", [128, 1], f32, kind="ExternalInput")
    bv = nc.dram_tensor("bv", [128, 1], f32, kind="ExternalInput")
    bp = nc.dram_tensor("bp", [128, D], f32, kind="ExternalInput")
    ident = nc.dram_tensor("ident", [128, 128], bf16, kind="ExternalInput")
    identf = nc.dram_tensor("identf", [128, 128], f32, kind="ExternalInput")
    # rows: (b, pair, i) with i in [0,128): j = 2*pair + i//64, own-row i%64
    out = nc.dram_tensor("out", [B * 2 * 128, D], f32, kind="ExternalOutput")
    dbg = (
        nc.dram_tensor("dbg", [B * NTQB * NLOC * H, TQB], bf16, kind="ExternalOutput")
        if DEBUG_AN
        else None
    )
    dbg2 = (
        nc.dram_tensor("dbg2", [4 * 128, NCORES * 128], bf16, kind="ExternalOutput")
        if DEBUG_AN
        else None
    )

    with tile.TileContext(nc) as tc, ExitStack() as ctx:
        p_const = ctx.enter_context(tc.tile_pool(name="const", bufs=1))
        p_x = ctx.enter_context(tc.tile_pool(name="x", bufs=1))
        p_qk = ctx.enter_context(tc.tile_pool(name="qk", bufs=2))
        p_va = ctx.enter_context(tc.tile_pool(name="va", bufs=2))
        p_pt = ctx.enter_context(tc.tile_pool(name="pt", bufs=3))
        p_a = ctx.enter_context(tc.tile_pool(name="a", bufs=2))
        p_o = ctx.enter_context(tc.tile_pool(name="o", bufs=2))
        p_dram = ctx.enter_context(tc.tile_pool(name="dram", bufs=1, space="DRAM"))

        ps_ss = ctx.enter_context(tc.tile_pool(name="ps_ss", bufs=1, space="PSUM"))
        ps_pv = ctx.enter_context(tc.tile_pool(name="ps_pv", bufs=1, space="PSUM"))
        ps_mm = ctx.enter_context(tc.tile_pool(name="ps_mm", bufs=2, space="PSUM"))

        # ---- constant tiles -------------------------------------------------
        wq_sb = p_const.tile([128, NDC * NW], bf16)
        wk_sb = p_const.tile([128, NDC * NW], bf16)
        wv_sb = p_const.tile([128, NDC * NW], bf16)
        wp_sb = p_const.tile([128, (N_HEADS * H // 128) * D], bf16)
        bq_sb = p_const.tile([128, 1], f32)
        bk_sb = p_const.tile([128, 1], f32)
        bv_sb = p_const.tile([128, 1], f32)
        bp_sb = p_const.tile([128, D], f32)
        id_sb = p_const.tile([128, 128], bf16)
        idf_sb = p_const.tile([128, 128], f32)

        # warm the ACT exp table while everything else is still loading
        warm = p_const.tile([1, 8], bf16)
        nc.vector.memset(warm[:], 0.0)
        nc.scalar.activation(warm[:], warm[:], AF.Exp)

        # ones tiles: f32 (K=1 transpose "identity") and bf16 (broadcast
        # outer-product column, 1 col/cycle instead of fp32's 1/2)
        onesf = p_const.tile([128, H], f32)
        nc.vector.memset(onesf[:], 1.0)
        onesb = p_const.tile([128, H], bf16)
        nc.vector.memset(onesb[:], 1.0)

        # ---- HAM pre-warm: keep the PE busy from t~0 so the clock gate is
        # released (1.2 -> 2.4 GHz) before the first real projection matmul.
        if WARMUP:
            wtile = p_const.tile([128, 64], bf16)
            nc.vector.memset(wtile[:], 0.0)
            wps = ps_mm.tile([64, 64], f32, name="wps", tag="mm")
            for _ in range(WARMUP):
                nc.tensor.matmul(wps[:], lhsT=wtile[:, 0:64], rhs=wtile[:], start=True, stop=True)

        BLOCKS = [(b, j) for b in range(B) for j in range(NTQB)]
        # Pair-level exchange buffers: [8 src cores x 128 headdim,
        # (half, own-64)] per (batch, pair-of-blocks). 4 collectives of 256KB
        # each — the mesh AllToAll has ~15-30us fixed cost per op, so
        # per-block (8x128KB) exchanges serialize into the critical path.
        # NOTE: untagged tiles in one pool share a single slot ring — every
        # DRAM tile needs its own tag or all collectives alias one buffer.
        PAIRS = [(b, pr) for b in range(B) for pr in range(2)]
        a2a_in = {
            pp: p_dram.tile(
                [NCORES * NW, 2 * OWN], bf16,
                name=f"a2ai{pp[0]}{pp[1]}", tag=f"a2ai{pp[0]}{pp[1]}",
            )
            for pp in PAIRS
        }
        a2a_out = {
            pp: p_dram.tile(
                [NCORES * NW, 2 * OWN], bf16,
                name=f"a2ao{pp[0]}{pp[1]}", tag=f"a2ao{pp[0]}{pp[1]}",
            )
            for pp in PAIRS
        }

        # ---- x loads: per (tensor, tb) contiguous 8KB-line chunks -----------
        CW = NDC * XB  # 4096 cols per tb chunk
        exts = {"v": vT, "k": kT, "q": qT}
        xs = {0: {}, 1: {}}

        def load_x_one(b, tname, tb):
            # q tiles ride a 2-buffer ring (q2/q3 arrive ~25us after Q0/Q1
            # projected — no WAR stall); k/v keep per-tb buffers so the
            # in-order load queue never waits on a projection mid-stream.
            tg = f"x{tname}{tb % 2}" if tname == "q" else f"x{tname}{tb}"
            t_ = p_x.tile([128, CW], bf16, name=f"x{tname}{b}{tb}", tag=tg)
            nc.sync.dma_start(t_[:], exts[tname][b][:, tb * CW : (tb + 1) * CW])
            xs[b][(tname, tb)] = t_

        def load_startup():
            """Loads in consumption order. b=0: k0,q0 first (scores start as
            soon as K0/Q0 project), then k/v interleaved at the rate block
            (0,0) consumes chunks, then q1-3 (needed from block (0,1)).
            b=1 x-loads reuse b=0's buffers (same tag -> WAR on b0 proj).
            wp (2MB) before b1 q-loads: first outproj filler fires ~85us in."""
            nc.sync.dma_start(id_sb[:], ident[:])
            nc.sync.dma_start(wk_sb[:], wk[:])
            nc.sync.dma_start(bk_sb[:], bk[:])
            load_x_one(0, "k", 0)
            nc.sync.dma_start(wq_sb[:], wq[:])
            nc.sync.dma_start(bq_sb[:], bq[:])
            load_x_one(0, "q", 0)
            load_x_one(0, "k", 1)
            nc.sync.dma_start(wv_sb[:], wv[:])
            nc.sync.dma_start(bv_sb[:], bv[:])
            load_x_one(0, "v", 0)
            load_x_one(0, "v", 1)
            load_x_one(0, "k", 2)
            load_x_one(0, "v", 2)
            load_x_one(0, "k", 3)
            load_x_one(0, "q", 1)
            load_x_one(0, "v", 3)
            load_x_one(0, "q", 2)
            load_x_one(0, "q", 3)
            nc.sync.dma_start(idf_sb[:], identf[:])
            nc.sync.dma_start(bp_sb[:], bp[:])
            for tb in range(NTQB):
                load_x_one(1, "v", tb)
                load_x_one(1, "k", tb)
            nc.sync.dma_start(wp_sb[:], wp[:])
            for tb in range(NTQB):
                load_x_one(1, "q", tb)

        # ---- projection pieces (emitted inline or as PE fillers) ------------
        def proj_block(w_sb, b_sb, xt, dst, dcol, b, tag):
            ps = ps_mm.tile([128, XB], f32, name=f"mm{tag}{b}{dcol}", tag="mm")
            for dc in range(NDC):
                nc.tensor.matmul(
                    ps[:],
                    lhsT=w_sb[:, dc * NW : (dc + 1) * NW],
                    rhs=xt[:, dc * XB : (dc + 1) * XB],
                    start=(dc == 0),
                    stop=(dc == NDC - 1),
                )
            nc.vector.tensor_scalar(
                dst[:, dcol * XB : (dcol + 1) * XB], ps[:], b_sb[:, 0:1], None,
                ALU.add,
            )

        def va_piece(b, vt, va, i0, n):
            """Transpose tk-chunks [i0, i0+n) of vt into va."""
            for i in range(i0, i0 + n):
                pst = ps_mm.tile([128, 128], bf16, name=f"pst{b}{i}", tag="mm")
                nc.tensor.transpose(pst[:], vt[:, i * TKC : (i + 1) * TKC], id_sb[:])
                dst = va[:, i * NLOC * VA : (i + 1) * NLOC * VA].rearrange(
                    "p (h x) -> p h x", x=VA
                )[:, :, 0:H]
                nc.vector.tensor_copy(dst, pst[:].rearrange("p (h x) -> p h x", x=H))

        def make_proj(b):
            """Returns (qt, kt, va, pieces): pieces in pipeline-consumption
            order (K first, then V/va interleaved, Q last)."""
            vt = p_va.tile([128, T], bf16, name=f"vt{b}", tag="vt")
            va = p_va.tile([128, NTKC * NLOC * VA], bf16, name=f"va{b}", tag="va")
            qt = p_qk.tile([128, T], bf16, name=f"qt{b}", tag="qt")
            kt = p_qk.tile([128, T], bf16, name=f"kt{b}", tag="kt")
            nc.vector.memset(
                va[:].rearrange("p (i h x) -> p i h x", h=NLOC, x=VA)[
                    :, :, :, H : H + 1
                ],
                1.0,
            )
            mk = lambda w, bb, tn, dst, tb, tg: (
                lambda: proj_block(w, bb, xs[b][(tn, tb)], dst, tb, b, tg)
            )
            # consumption order matches the DMA arrival order (~1 chunk per
            # exp group): V tb / va / K tb interleaved, Q blocks last.
            pieces = [
                mk(wk_sb, bk_sb, "k", kt, 0, "k"),
                mk(wq_sb, bq_sb, "q", qt, 0, "q"),
                mk(wv_sb, bv_sb, "v", vt, 0, "v"),
                lambda: va_piece(b, vt, va, 0, 4),
                mk(wk_sb, bk_sb, "k", kt, 1, "k"),
                mk(wv_sb, bv_sb, "v", vt, 1, "v"),
                lambda: va_piece(b, vt, va, 4, 4),
                mk(wk_sb, bk_sb, "k", kt, 2, "k"),
                mk(wv_sb, bv_sb, "v", vt, 2, "v"),
                lambda: va_piece(b, vt, va, 8, 4),
                mk(wk_sb, bk_sb, "k", kt, 3, "k"),
                mk(wv_sb, bv_sb, "v", vt, 3, "v"),
                lambda: va_piece(b, vt, va, 12, 4),
                mk(wq_sb, bq_sb, "q", qt, 1, "q"),
                mk(wq_sb, bq_sb, "q", qt, 2, "q"),
                mk(wq_sb, bq_sb, "q", qt, 3, "q"),
            ]
            return qt, kt, va, pieces

        fillers = []

        def run_filler(n=1):
            for _ in range(n):
                if fillers:
                    fillers.pop(0)()

        # ---- output projection for a PAIR of blocks -------------------------
        # ats_pair[:, s*128 + half*64 : +64] holds src-core s's heads for the
        # half-th block of the pair; one [128,512] matmul group per D-half.
        def outproj_pair(b, pr, ats_pair):
            def op(dh):
                ps = ps_mm.tile([128, 512], f32, name=f"po{b}{pr}{dh}", tag="mm")
                for s in range(NCORES):
                    nc.tensor.matmul(
                        ps[:],
                        lhsT=ats_pair[:, s * 128 : (s + 1) * 128],
                        rhs=wp_sb[:, s * D + dh * 512 : s * D + (dh + 1) * 512],
                        start=(s == 0),
                        stop=(s == NCORES - 1),
                    )
                o_sb = p_o.tile([128, 512], f32, name=f"o{b}{pr}{dh}", tag="o")
                nc.vector.tensor_tensor(
                    o_sb[:], ps[:], bp_sb[:, dh * 512 : (dh + 1) * 512], ALU.add
                )
                nc.sync.dma_start(
                    out[
                        (b * 2 + pr) * 128 : (b * 2 + pr + 1) * 128,
                        dh * 512 : (dh + 1) * 512,
                    ],
                    o_sb[:],
                )
            return [lambda dh=dh: op(dh) for dh in range(2)]

        def ats_load(b, pr, ats_pair):
            for s in range(NCORES):
                nc.sync.dma_start(
                    ats_pair[:, s * 128 : (s + 1) * 128],
                    a2a_out[(b, pr)][s * 128 : (s + 1) * 128, :],
                )

        # ---- attention pieces ----------------------------------------------
        st = {}  # live per-block state: pv tiles, pss, pt, a_sb

        def scores(bl, g, qt, kt):
            # head-major: each head's chunk pair releases together (gated by
            # its exp) and the two MMs pipeline fill-under-drain.
            b, j = bl
            for hd in range(NLOC):
                key = (bl, g % 2, hd)
                st[("ss",) + key] = ps_ss.tile(
                    [128, RG * TQB], f32,
                    name=f"pss{b}{j}{g}{hd}", tag=f"ss{hd}",
                )
                for r in range(g * RG, (g + 1) * RG):
                    nc.tensor.matmul(
                        st[("ss",) + key][:, (r % RG) * TQB : (r % RG + 1) * TQB],
                        lhsT=kt[hd * H : (hd + 1) * H, r * TKC : (r + 1) * TKC],
                        rhs=qt[hd * H : (hd + 1) * H, j * TQB : (j + 1) * TQB],
                        start=True,
                        stop=True,
                    )

        def exps(bl, g):
            b, j = bl
            for hd in range(NLOC):
                pt_t = p_pt.tile(
                    [128, RG * TQB], bf16,
                    name=f"pt{b}{j}{g}{hd}", tag=f"pt{hd}",
                )
                st[("pt", bl, g % 2, hd)] = pt_t
                nc.scalar.activation(
                    pt_t[:], st[("ss", bl, g % 2, hd)][:], AF.Exp
                )

        def pvs(bl, g, va):
            b, j = bl
            if g == 0:
                st[("pv", bl)] = [
                    ps_pv.tile([VA, TQB], f32, name=f"pv{b}{j}{hd}", tag=f"pv{hd}")
                    for hd in range(NLOC)
                ]
            pv = st[("pv", bl)]
            for hd in range(NLOC):
                for r in range(g * RG, (g + 1) * RG):
                    col0 = (r * NLOC + hd) * VA
                    nc.tensor.matmul(
                        pv[hd][:],
                        lhsT=va[:, col0 : col0 + VA],
                        rhs=st[("pt", bl, g % 2, hd)][:, (r % RG) * TQB : (r % RG + 1) * TQB],
                        start=(g == 0 and r == g * RG),
                        stop=(g == NG - 1 and r == (g + 1) * RG - 1),
                    )

        def norm_a(bl):
            """Evacuate pv -> SBUF (frees the pv PSUM banks for the next
            block's accumulation)."""
            b, j = bl
            pv = st.pop(("pv", bl))
            for hd in range(NLOC):
                a_sb = p_a.tile(
                    [H + 1, TQB], f32, name=f"as{b}{j}{hd}", tag=f"as{hd}"
                )
                nc.vector.tensor_copy(a_sb[:], pv[hd][0 : H + 1, :])
                st[("as", bl, hd)] = a_sb

        def norm_b(bl, hd):
            """Reciprocal of the denominator row (PE-transposed to [128,4] so
            the DVE op is cheap), broadcast to H rows, multiply, stage into
            the pair's AllToAll input. (Custom DVE ops are unusable: this
            toolchain's walrus rejects InstCustomDveAnt.)"""
            b, j = bl
            NTR = TQB // 128  # 4
            a_sb = st.pop(("as", bl, hd))
            trp = ps_mm.tile([128, NTR], f32, name=f"trp{b}{j}{hd}", tag="mm")
            for i in range(NTR):
                nc.tensor.transpose(
                    trp[:, i : i + 1],
                    a_sb[H : H + 1, i * 128 : (i + 1) * 128],
                    onesf[H : H + 1, 0:1],
                )
            rc = p_a.tile([128, NTR], f32, name=f"rc{b}{j}{hd}", tag=f"rc{hd}")
            nc.vector.reciprocal(rc[:], trp[:])
            rowt = ps_mm.tile([1, TQB], f32, name=f"rw{b}{j}{hd}", tag="mm")
            for i in range(NTR):
                nc.tensor.transpose(
                    rowt[:, i * 128 : (i + 1) * 128],
                    rc[:, i : i + 1],
                    idf_sb[:, 0:128],
                )
            rr = p_a.tile([1, TQB], bf16, name=f"rr{b}{j}{hd}", tag=f"rr{hd}")
            nc.vector.tensor_copy(rr[:], rowt[:])
            rep_ps = ps_mm.tile([H, TQB], f32, name=f"rp{b}{j}{hd}", tag="mm")
            nc.tensor.matmul(
                rep_ps[:], lhsT=onesb[0:1, 0:H], rhs=rr[:], start=True, stop=True
            )
            an = p_a.tile([H, TQB], bf16, name=f"an{b}{j}{hd}", tag=f"an{hd}")
            nc.vector.tensor_tensor(an[:], a_sb[0:H, :], rep_ps[:], ALU.mult)
            if dbg is not None:
                r0 = ((b * NTQB + j) * NLOC + hd) * H
                nc.sync.dma_start(dbg[r0 : r0 + H, :], an[:])
            # stage into the pair buffer: rows c*128 + hd*64 + h, cols
            # (j%2)*64 + own-64 of tq. Iteration order (h, c, t) on both
            # sides; SBUF AP keeps the partition dim (h) first.
            half = j % 2
            dst = a2a_in[(b, j // 2)].rearrange("(c s) t -> s c t", s=NW)[
                hd * H : (hd + 1) * H, :, half * OWN : (half + 1) * OWN
            ]
            src = an[:].rearrange("h (c t) -> h c t", t=OWN)
            # SP queue: behind the x-loads (first ~75us), which is fine — the
            # collective chain is trigger-paced after that. (scalar/gpsimd
            # DMA queues corrupt this strided transfer: NaN / wrong layout.)
            nc.sync.dma_start(dst, src)

        def collective(pp):
            nc.gpsimd.collective_compute(
                "AllToAll",
                mybir.AluOpType.bypass,
                replica_groups=GROUPS,
                ins=[a2a_in[pp].opt()],
                outs=[a2a_out[pp].opt()],
            )

        # ===== main schedule =================================================
        load_startup()
        qt0, kt0, va0, pieces0 = make_proj(0)
        # inline: K0, Q0 only -- scores start as soon as they project.
        pieces0[0]()
        pieces0[1]()
        # b0 filler order = consumption order (V0/K1/va03 lead; Q1/Q2 hoisted
        # so each lands >=1 group before its block's first scores).
        fillers.extend(
            [pieces0[i] for i in (2, 4, 3, 7, 5, 6, 8, 10, 9, 13, 11, 12, 14, 15)]
        )

        qt1, kt1, va1, p1 = make_proj(1)
        # b1 consumption order matching its v/k-interleaved DMA arrival
        fillers.extend(
            [p1[i] for i in (2, 0, 3, 5, 4, 6, 8, 7, 9, 11, 10, 12, 1, 13, 14, 15)]
        )

        # EMISSION ORDER IS DATAFLOW: a filler emitted after its consumer's
        # emission silently reads stale data. Pops per (bi, g), hand-paced to
        # DMA arrival while keeping >=1 group of margin before each consumer.
        # bi=0 list: [V0, K1, va03 | - | K2, V1, va47 | V2 | K3, va811 | Q1 |
        #             V3, va1215, Q2 | -]
        POPS = {
            0: [3, 0, 3, 1, 2, 1, 3, 0],
            1: [0, 0, 0, 2, 1, 2, 1, 1],
            2: [0, 0, 0, 1, 1, 1, 1, 1],
            3: [0, 0, 0, 1, 1, 1, 1, 1],
        }
        DEF_POPS = [0, 0, 0, 0, 2, 2, 2, 2]

        qkv = {0: (qt0, kt0, va0), 1: (qt1, kt1, va1)}
        ats_pairs = {}
        for b in range(B):
            for pr in range(2):
                ats_pairs[(b, pr)] = p_a.tile(
                    [128, NCORES * 128], bf16, name=f"ats{b}{pr}", tag=f"ats{pr}"
                )

        NB = len(BLOCKS)
        scores(BLOCKS[0], 0, qt0, kt0)
        exps(BLOCKS[0], 0)
        for bi, bl in enumerate(BLOCKS):
            b, j = bl
            qt, kt, va = qkv[b]
            for g in range(NG):
                # one-group lookahead (crosses block boundaries)
                if g + 1 < NG:
                    scores(bl, g + 1, qt, kt)
                    exps(bl, g + 1)
                elif bi + 1 < NB:
                    nbl = BLOCKS[bi + 1]
                    nqt, nkt, _ = qkv[nbl[0]]
                    scores(nbl, 0, nqt, nkt)
                    exps(nbl, 0)
                if bi > 0:
                    pbl = BLOCKS[bi - 1]
                    if g == 0:
                        norm_b(pbl, 0)
                    elif g == 1:
                        norm_b(pbl, 1)
                    elif g == 2 and pbl[1] % 2 == 1:
                        collective((pbl[0], pbl[1] // 2))
                    elif g == 3 and bi in (4, 6):
                        # ats for the pair exchanged TWO blocks ago (its
                        # collective is long done -> no SP-queue stall)
                        pb, pj = BLOCKS[bi - 3]
                        pr = pj // 2
                        ats_load(pb, pr, ats_pairs[(pb, pr)])
                        fillers.extend(outproj_pair(pb, pr, ats_pairs[(pb, pr)]))
                run_filler(POPS.get(bi, DEF_POPS)[g])
                pvs(bl, g, va)
                if g == NG - 1:
                    norm_a(bl)

        # ---- tail: last block's norm + pair collectives + outprojs ---------
        last = BLOCKS[-1]
        norm_b(last, 0)
        norm_b(last, 1)
        collective((1, 1))
        # pair (1,0)'s collective was triggered two blocks ago; its outproj
        # runs while pair (1,1)'s collective is in flight.
        ats_load(1, 0, ats_pairs[(1, 0)])
        for p in outproj_pair(1, 0, ats_pairs[(1, 0)]):
            p()
        run_filler(len(fillers))
        ats_load(1, 1, ats_pairs[(1, 1)])
        for p in outproj_pair(1, 1, ats_pairs[(1, 1)]):
            p()
        if dbg2 is not None:
            for b in range(B):
                for pr in range(2):
                    nc.sync.dma_start(
                        dbg2[(b * 2 + pr) * 128 : (b * 2 + pr + 1) * 128, :],
                        ats_pairs[(b, pr)][:],
                    )

    orig_to_json = nc.to_json_bytes
    nc.to_json_bytes = lambda: _legalize_waits(orig_to_json())
    return nc


def _get_nc():
    if "nc" not in _CACHE:
        _CACHE["nc"] = _build()
    return _CACHE["nc"]


def _make_in_maps(inputs):
    q = np.asarray(inputs["q"], dtype=np.float32)
    v = np.asarray(inputs["v"], dtype=np.float32)
    k = np.asarray(inputs["k"], dtype=np.float32)
    w_query = np.asarray(inputs["w_query"], dtype=np.float32)
    b_query = np.asarray(inputs["b_query"], dtype=np.float32)
    w_value = np.asarray(inputs["w_value"], dtype=np.float32)
    b_value = np.asarray(inputs["b_value"], dtype=np.float32)
    w_key = np.asarray(inputs["w_key"], dtype=np.float32)
    b_key = np.asarray(inputs["b_key"], dtype=np.float32)
    w_projection = np.asarray(inputs["w_projection"], dtype=np.float32)
    b_projection = np.asarray(inputs["b_projection"], dtype=np.float32)

    scale = np.float32(1.0 / np.sqrt(H))

    def arrange_w(w):
        # [D or N*H, m] -> SBUF layout [128, (chunk, m)], contiguous rows
        m = w.shape[1]
        return np.ascontiguousarray(
            w.reshape(-1, 128, m).transpose(1, 0, 2).reshape(128, -1)
        ).astype(BF16)

    def arrange_x(xb):
        # [T, D] -> [D, T] -> SBUF layout [128, (tb, dc, t)], contiguous rows
        return np.ascontiguousarray(
            xb.T.reshape(NDC, 128, NTQB, XB)
            .transpose(1, 2, 0, 3)
            .reshape(128, NTQB * NDC * XB)
        ).astype(BF16)

    wp_s = arrange_w(
        np.ascontiguousarray(
            w_projection.transpose(0, 2, 1).reshape(N_HEADS * H, D)
        )
    )
    bp_s = np.ascontiguousarray(
        np.tile(b_projection.reshape(1, D), (128, 1))
    ).astype(np.float32)

    xT = {}
    for b in range(B):
        xT[b] = tuple(arrange_x(x[b]) for x in (q, k, v))

    in_maps = []
    for c in range(NCORES):
        hs = c * NLOC
        wq_s = arrange_w(w_query[:, hs : hs + NLOC, :].reshape(D, NW) * scale)
        wk_s = arrange_w(w_key[:, hs : hs + NLOC, :].reshape(D, NW))
        wv_s = arrange_w(w_value[:, hs : hs + NLOC, :].reshape(D, NW))
        bq_s = np.ascontiguousarray(
            (b_query[hs : hs + NLOC].reshape(NW) * scale).reshape(NW, 1)
        ).astype(np.float32)
        bk_s = np.ascontiguousarray(
            b_key[hs : hs + NLOC].reshape(NW, 1)
        ).astype(np.float32)
        bv_s = np.ascontiguousarray(
            b_value[hs : hs + NLOC].reshape(NW, 1)
        ).astype(np.float32)
        m = {
            "ident": np.eye(128, dtype=np.float32).astype(BF16),
            "identf": np.eye(128, dtype=np.float32),
            "wq": np.ascontiguousarray(wq_s),
            "wk": np.ascontiguousarray(wk_s),
            "wv": np.ascontiguousarray(wv_s),
            "wp": wp_s,
            "bq": bq_s,
            "bk": bk_s,
            "bv": bv_s,
            "bp": bp_s,
        }
        for b in range(B):
            m[f"qT{b}"], m[f"kT{b}"], m[f"vT{b}"] = xT[b]
        in_maps.append(m)
    return in_maps


def _assemble(results):
    out = np.empty((B, T, D), np.float32)
    for c in range(NCORES):
        res = results[c]["out"]  # [B*2*128, D]: rows (b, pair, i)
        for b in range(B):
            for pr in range(2):
                blk = res[(b * 2 + pr) * 128 : (b * 2 + pr + 1) * 128]
                for half in range(2):
                    j = 2 * pr + half
                    r0 = j * TQB + c * OWN
                    out[b, r0 : r0 + OWN, :] = blk[half * OWN : (half + 1) * OWN]
    return out


def run(inputs, trace=False, **kwargs):
    from concourse.bass_utils import run_bass_kernel_spmd

    nc = _get_nc()
    in_maps = _make_in_maps(inputs)
    res = run_bass_kernel_spmd(
        nc, in_maps, list(range(NCORES)), trace=trace, **kwargs
    )
    return _assemble(res.results), res


def kernel(**inputs) -> np.ndarray:
    out, _ = run(inputs, trace=False)
    return out


# revision 38
# speedup vs baseline: 1.0756x; 1.0249x over previous
"""Bass/Tile TRN2 kernel: 16-head MHA (B=2, T=2048, D=1024, H=64) on 8 NeuronCores.

Sharding: 8-way tensor parallel over heads — core c computes heads {2c, 2c+1}
for BOTH batches. Output ownership: within every (batch, 512-row tq block),
core c owns the 64 rows [c*64, (c+1)*64). After each block's attention one
small (128KB) AllToAll exchanges head-shards for row-shards, so every
collective except the last overlaps later attention blocks, and the output
projection for a PAIR of blocks (2x64 owned rows = 128 partitions) runs as
PE filler work.

Per-core pipeline (bf16 into the PE, fp32 PSUM accumulation):
  - ~200 tiny warm-up matmuls at t=0 trip the PE HAM clock gate to 2.4 GHz
    before the first projection.
  - DMA order: k0,q0 first so scores start ~15us in; k/v interleaved next at
    the rate the attention pipeline consumes them; wp before the first
    output-projection filler is needed.
  - QKV projections: 8x [128,128]x[128,512] accumulating matmuls per block.
  - Scores S^T[tk, tq] = K^T.T @ Q^T per head ([64,128] stationary, auto
    row-group packing); 1/sqrt(H) folded into Wq/bq on host.
  - exp on ScalarE straight out of PSUM in 1024-wide ACTIVATEs; the two
    heads' score PSUM buffers alternate so ACT never waits on score matmuls.
    Attention is emitted as ONE flat software pipeline over all 8 (b, tq)
    blocks: the next block's scores/exps are emitted before the previous
    block's normalize, so ACT stays saturated across block boundaries.
  - PV matmul with a ones-augmented V (stationary col 64 = ones) so row 64 of
    the PV accumulator is the softmax denominator for free.
  - Normalize: pv evacuated by DVE (frees the PSUM bank), denominator row
    PE-transposed to [128,4] for a cheap DVE reciprocal, broadcast back via a
    1-row outer-product matmul, multiply, stage into the block's AllToAll
    buffer as [8 dest cores x 128 headdim, 64 tq].
  - PE idle slots inside the ACT-bound phase are filled with the next batch's
    projections and completed pairs' output projections.
Host does layout-only prep (transpose, bf16 cast, weight slicing) and
scatters the 8 cores' per-block 64-row output slices.
"""

import sys
from contextlib import ExitStack

import numpy as np

sys.path.insert(0, "/opt/trn_rl_repo")

import ml_dtypes  # noqa: E402

BF16 = ml_dtypes.bfloat16

B, T, D = 2, 2048, 1024
N_HEADS, H = 16, 64
NCORES = 8
GROUPS = [[0, 1, 2, 3, 4, 5, 6, 7]]
NLOC = 2            # heads per core
TQB = 512           # attention tq block
NTQB = T // TQB     # 4
TKC = 128           # tk chunk
NTKC = T // TKC     # 16
RG = 2              # tk chunks per exp group
NG = NTKC // RG     # 8 groups per block
DC = 128            # d chunk
NDC = D // DC       # 8
XB = 512            # x-load column block
OWN = 64            # tq rows per core per block
VA = 128            # V_aug stationary width: [V(64) | ones(1) | junk(63)]
NW = NLOC * H       # 128 projection width per core
WARMUP = 200        # HAM pre-warm matmul count (0 = off)
DEBUG_AN = False    # dump normalized attention tiles to a debug output

_CACHE = {}


def _legalize_waits(bir_bytes):
    """This toolchain's walrus accepts at most ONE semaphore wait per
    instruction ("Too many sync wait commands"). Tile's sem assignment emits
    several. Hoist all but one wait of each instruction onto same-engine NoOps
    inserted immediately before it (engines execute their stream in order, so
    waiting earlier on the same engine is equivalent)."""
    import json

    j = json.loads(bir_bytes)
    ctr = 0
    for fn in j["functions"]:
        for blk in fn["blocks"]:
            out = []
            for ins in blk["instructions"]:
                si = ins.get("sync_info")
                waits = (si or {}).get("on_wait") or []
                if len(waits) > 1:
                    for w in waits[:-1]:
                        ctr += 1
                        out.append(
                            {
                                "engine": ins["engine"],
                                "ins": [],
                                "outs": [],
                                "name": f"waitfix-{ctr}",
                                "opcode": "NoOp",
                                "sync_info": {"on_wait": [w], "on_update": []},
                            }
                        )
                    si["on_wait"] = [waits[-1]]
                out.append(ins)
            blk["instructions"] = out
    return json.dumps(j).encode()


def _build():
    import concourse.bass as bass
    import concourse.mybir as mybir
    import concourse.tile as tile

    f32 = mybir.dt.float32
    bf16 = mybir.dt.bfloat16
    AF = mybir.ActivationFunctionType
    ALU = mybir.AluOpType

    nc = bass.Bass(
        "TRN2", target_bir_lowering=False, debug=False, num_devices=NCORES
    )

    # activations/weights arrive pre-arranged on host into the exact SBUF
    # layout ([partition, ...] contiguous) so every load is a 1:1 DMA with
    # 2KB+ lines and 128 descriptors.
    qT = [nc.dram_tensor(f"qT{b}", [128, NTQB * NDC * XB], bf16, kind="ExternalInput") for b in range(B)]
    kT = [nc.dram_tensor(f"kT{b}", [128, NTQB * NDC * XB], bf16, kind="ExternalInput") for b in range(B)]
    vT = [nc.dram_tensor(f"vT{b}", [128, NTQB * NDC * XB], bf16, kind="ExternalInput") for b in range(B)]
    wq = nc.dram_tensor("wq", [128, NDC * NW], bf16, kind="ExternalInput")
    wk = nc.dram_tensor("wk", [128, NDC * NW], bf16, kind="ExternalInput")
    wv = nc.dram_tensor("wv", [128, NDC * NW], bf16, kind="ExternalInput")
    wp = nc.dram_tensor("wp", [128, (N_HEADS * H // 128) * D], bf16, kind="ExternalInput")
    bq = nc.dram_tensor("bq", [128, 1], f32, kind="ExternalInput")
    bk = nc.dram_tensor("bk", [128, 1], f32, kind="ExternalInput")
    bv = nc.dram_tensor("bv", [128, 1], f32, kind="ExternalInput")
    bp = nc.dram_tensor("bp", [128, D], f32, kind="ExternalInput")
    ident = nc.dram_tensor("ident", [128, 128], bf16, kind="ExternalInput")
    identf = nc.dram_tensor("identf", [128, 128], f32, kind="ExternalInput")
    # rows: (b, pair, i) with i in [0,128): j = 2*pair + i//64, own-row i%64
    out = nc.dram_tensor("out", [B * 2 * 128, D], f32, kind="ExternalOutput")
    dbg = (
        nc.dram_tensor("dbg", [B * NTQB * NLOC * H, TQB], bf16, kind="ExternalOutput")
        if DEBUG_AN
        else None
    )
    dbg2 = (
        nc.dram_tensor("dbg2", [4 * 128, NCORES * 128], bf16, kind="ExternalOutput")
        if DEBUG_AN
        else None
    )

    with tile.TileContext(nc) as tc, ExitStack() as ctx:
        p_const = ctx.enter_context(tc.tile_pool(name="const", bufs=1))
        p_x = ctx.enter_context(tc.tile_pool(name="x", bufs=1))
        p_qk = ctx.enter_context(tc.tile_pool(name="qk", bufs=2))
        p_va = ctx.enter_context(tc.tile_pool(name="va", bufs=2))
        p_pt = ctx.enter_context(tc.tile_pool(name="pt", bufs=3))
        p_a = ctx.enter_context(tc.tile_pool(name="a", bufs=2))
        p_o = ctx.enter_context(tc.tile_pool(name="o", bufs=2))
        p_dram = ctx.enter_context(tc.tile_pool(name="dram", bufs=1, space="DRAM"))

        ps_ss = ctx.enter_context(tc.tile_pool(name="ps_ss", bufs=1, space="PSUM"))
        ps_pv = ctx.enter_context(tc.tile_pool(name="ps_pv", bufs=1, space="PSUM"))
        ps_mm = ctx.enter_context(tc.tile_pool(name="ps_mm", bufs=2, space="PSUM"))

        # ---- constant tiles -------------------------------------------------
        wq_sb = p_const.tile([128, NDC * NW], bf16)
        wk_sb = p_const.tile([128, NDC * NW], bf16)
        wv_sb = p_const.tile([128, NDC * NW], bf16)
        wp_sb = p_const.tile([128, (N_HEADS * H // 128) * D], bf16)
        bq_sb = p_const.tile([128, 1], f32)
        bk_sb = p_const.tile([128, 1], f32)
        bv_sb = p_const.tile([128, 1], f32)
        bp_sb = p_const.tile([128, D], f32)
        id_sb = p_const.tile([128, 128], bf16)
        idf_sb = p_const.tile([128, 128], f32)

        # warm the ACT exp table while everything else is still loading
        warm = p_const.tile([1, 8], bf16)
        nc.vector.memset(warm[:], 0.0)
        nc.scalar.activation(warm[:], warm[:], AF.Exp)

        # ones tiles: f32 (K=1 transpose "identity") and bf16 (broadcast
        # outer-product column, 1 col/cycle instead of fp32's 1/2)
        onesf = p_const.tile([128, H], f32)
        nc.vector.memset(onesf[:], 1.0)
        onesb = p_const.tile([128, H], bf16)
        nc.vector.memset(onesb[:], 1.0)

        # ---- HAM pre-warm: keep the PE busy from t~0 so the clock gate is
        # released (1.2 -> 2.4 GHz) before the first real projection matmul.
        if WARMUP:
            wtile = p_const.tile([128, 64], bf16)
            nc.vector.memset(wtile[:], 0.0)
            wps = ps_mm.tile([64, 64], f32, name="wps", tag="mm")
            for _ in range(WARMUP):
                nc.tensor.matmul(wps[:], lhsT=wtile[:, 0:64], rhs=wtile[:], start=True, stop=True)

        BLOCKS = [(b, j) for b in range(B) for j in range(NTQB)]
        # Pair-level exchange buffers: [8 src cores x 128 headdim,
        # (half, own-64)] per (batch, pair-of-blocks). 4 collectives of 256KB
        # each — the mesh AllToAll has ~15-30us fixed cost per op, so
        # per-block (8x128KB) exchanges serialize into the critical path.
        # NOTE: untagged tiles in one pool share a single slot ring — every
        # DRAM tile needs its own tag or all collectives alias one buffer.
        PAIRS = [(b, pr) for b in range(B) for pr in range(2)]
        a2a_in = {
            pp: p_dram.tile(
                [NCORES * NW, 2 * OWN], bf16,
                name=f"a2ai{pp[0]}{pp[1]}", tag=f"a2ai{pp[0]}{pp[1]}",
            )
            for pp in PAIRS
        }
        a2a_out = {
            pp: p_dram.tile(
                [NCORES * NW, 2 * OWN], bf16,
                name=f"a2ao{pp[0]}{pp[1]}", tag=f"a2ao{pp[0]}{pp[1]}",
            )
            for pp in PAIRS
        }

        # ---- x loads: per (tensor, tb) contiguous 8KB-line chunks -----------
        CW = NDC * XB  # 4096 cols per tb chunk
        exts = {"v": vT, "k": kT, "q": qT}
        xs = {0: {}, 1: {}}

        def load_x_one(b, tname, tb):
            # q tiles ride a 2-buffer ring (q2/q3 arrive ~25us after Q0/Q1
            # projected — no WAR stall); k/v keep per-tb buffers so the
            # in-order load queue never waits on a projection mid-stream.
            tg = f"x{tname}{tb % 2}" if tname == "q" else f"x{tname}{tb}"
            t_ = p_x.tile([128, CW], bf16, name=f"x{tname}{b}{tb}", tag=tg)
            nc.sync.dma_start(t_[:], exts[tname][b][:, tb * CW : (tb + 1) * CW])
            xs[b][(tname, tb)] = t_

        def load_startup():
            """Loads in consumption order. b=0: k0,q0 first (scores start as
            soon as K0/Q0 project), then k/v interleaved at the rate block
            (0,0) consumes chunks, then q1-3 (needed from block (0,1)).
            b=1 x-loads reuse b=0's buffers (same tag -> WAR on b0 proj).
            wp (2MB) before b1 q-loads: first outproj filler fires ~85us in."""
            nc.sync.dma_start(id_sb[:], ident[:])
            nc.sync.dma_start(wk_sb[:], wk[:])
            nc.sync.dma_start(bk_sb[:], bk[:])
            load_x_one(0, "k", 0)
            nc.sync.dma_start(wq_sb[:], wq[:])
            nc.sync.dma_start(bq_sb[:], bq[:])
            load_x_one(0, "q", 0)
            load_x_one(0, "k", 1)
            nc.sync.dma_start(wv_sb[:], wv[:])
            nc.sync.dma_start(bv_sb[:], bv[:])
            load_x_one(0, "v", 0)
            load_x_one(0, "v", 1)
            load_x_one(0, "k", 2)
            load_x_one(0, "v", 2)
            load_x_one(0, "k", 3)
            load_x_one(0, "q", 1)
            load_x_one(0, "v", 3)
            load_x_one(0, "q", 2)
            load_x_one(0, "q", 3)
            nc.sync.dma_start(idf_sb[:], identf[:])
            nc.sync.dma_start(bp_sb[:], bp[:])
            for tb in range(NTQB):
                load_x_one(1, "v", tb)
                load_x_one(1, "k", tb)
            nc.sync.dma_start(wp_sb[:], wp[:])
            for tb in range(NTQB):
                load_x_one(1, "q", tb)

        # ---- projection pieces (emitted inline or as PE fillers) ------------
        def proj_block(w_sb, b_sb, xt, dst, dcol, b, tag):
            ps = ps_mm.tile([128, XB], f32, name=f"mm{tag}{b}{dcol}", tag="mm")
            for dc in range(NDC):
                nc.tensor.matmul(
                    ps[:],
                    lhsT=w_sb[:, dc * NW : (dc + 1) * NW],
                    rhs=xt[:, dc * XB : (dc + 1) * XB],
                    start=(dc == 0),
                    stop=(dc == NDC - 1),
                )
            nc.vector.tensor_scalar(
                dst[:, dcol * XB : (dcol + 1) * XB], ps[:], b_sb[:, 0:1], None,
                ALU.add,
            )

        def va_piece(b, vt, va, i0, n):
            """Transpose tk-chunks [i0, i0+n) of vt into va."""
            for i in range(i0, i0 + n):
                pst = ps_mm.tile([128, 128], bf16, name=f"pst{b}{i}", tag="mm")
                nc.tensor.transpose(pst[:], vt[:, i * TKC : (i + 1) * TKC], id_sb[:])
                dst = va[:, i * NLOC * VA : (i + 1) * NLOC * VA].rearrange(
                    "p (h x) -> p h x", x=VA
                )[:, :, 0:H]
                nc.vector.tensor_copy(dst, pst[:].rearrange("p (h x) -> p h x", x=H))

        def make_proj(b):
            """Returns (qt, kt, va, pieces): pieces in pipeline-consumption
            order (K first, then V/va interleaved, Q last)."""
            vt = p_va.tile([128, T], bf16, name=f"vt{b}", tag="vt")
            va = p_va.tile([128, NTKC * NLOC * VA], bf16, name=f"va{b}", tag="va")
            qt = p_qk.tile([128, T], bf16, name=f"qt{b}", tag="qt")
            kt = p_qk.tile([128, T], bf16, name=f"kt{b}", tag="kt")
            nc.vector.memset(
                va[:].rearrange("p (i h x) -> p i h x", h=NLOC, x=VA)[
                    :, :, :, H : H + 1
                ],
                1.0,
            )
            mk = lambda w, bb, tn, dst, tb, tg: (
                lambda: proj_block(w, bb, xs[b][(tn, tb)], dst, tb, b, tg)
            )
            # consumption order matches the DMA arrival order (~1 chunk per
            # exp group): V tb / va / K tb interleaved, Q blocks last.
            pieces = [
                mk(wk_sb, bk_sb, "k", kt, 0, "k"),
                mk(wq_sb, bq_sb, "q", qt, 0, "q"),
                mk(wv_sb, bv_sb, "v", vt, 0, "v"),
                lambda: va_piece(b, vt, va, 0, 4),
                mk(wk_sb, bk_sb, "k", kt, 1, "k"),
                mk(wv_sb, bv_sb, "v", vt, 1, "v"),
                lambda: va_piece(b, vt, va, 4, 4),
                mk(wk_sb, bk_sb, "k", kt, 2, "k"),
                mk(wv_sb, bv_sb, "v", vt, 2, "v"),
                lambda: va_piece(b, vt, va, 8, 4),
                mk(wk_sb, bk_sb, "k", kt, 3, "k"),
                mk(wv_sb, bv_sb, "v", vt, 3, "v"),
                lambda: va_piece(b, vt, va, 12, 4),
                mk(wq_sb, bq_sb, "q", qt, 1, "q"),
                mk(wq_sb, bq_sb, "q", qt, 2, "q"),
                mk(wq_sb, bq_sb, "q", qt, 3, "q"),
            ]
            return qt, kt, va, pieces

        fillers = []

        def run_filler(n=1):
            for _ in range(n):
                if fillers:
                    fillers.pop(0)()

        # ---- output projection for a PAIR of blocks -------------------------
        # ats_pair[:, s*128 + half*64 : +64] holds src-core s's heads for the
        # half-th block of the pair; one [128,512] matmul group per D-half.
        def outproj_pair(b, pr, ats_pair):
            def op(dh):
                ps = ps_mm.tile([128, 512], f32, name=f"po{b}{pr}{dh}", tag="mm")
                for s in range(NCORES):
                    nc.tensor.matmul(
                        ps[:],
                        lhsT=ats_pair[:, s * 128 : (s + 1) * 128],
                        rhs=wp_sb[:, s * D + dh * 512 : s * D + (dh + 1) * 512],
                        start=(s == 0),
                        stop=(s == NCORES - 1),
                    )
                o_sb = p_o.tile([128, 512], f32, name=f"o{b}{pr}{dh}", tag="o")
                nc.vector.tensor_tensor(
                    o_sb[:], ps[:], bp_sb[:, dh * 512 : (dh + 1) * 512], ALU.add
                )
                nc.sync.dma_start(
                    out[
                        (b * 2 + pr) * 128 : (b * 2 + pr + 1) * 128,
                        dh * 512 : (dh + 1) * 512,
                    ],
                    o_sb[:],
                )
            return [lambda dh=dh: op(dh) for dh in range(2)]

        def ats_load(b, pr, ats_pair):
            # one 256KB DMA: dst [p, s, c] (partition dim first), src matches
            nc.sync.dma_start(
                ats_pair[:].rearrange("p (s c) -> p s c", c=128),
                a2a_out[(b, pr)].rearrange("(s p) c -> p s c", p=128),
            )

        # ---- attention pieces ----------------------------------------------
        st = {}  # live per-block state: pv tiles, pss, pt, a_sb

        def scores(bl, g, qt, kt):
            # head-major: each head's chunk pair releases together (gated by
            # its exp) and the two MMs pipeline fill-under-drain.
            b, j = bl
            for hd in range(NLOC):
                key = (bl, g % 2, hd)
                st[("ss",) + key] = ps_ss.tile(
                    [128, RG * TQB], f32,
                    name=f"pss{b}{j}{g}{hd}", tag=f"ss{hd}",
                )
                for r in range(g * RG, (g + 1) * RG):
                    nc.tensor.matmul(
                        st[("ss",) + key][:, (r % RG) * TQB : (r % RG + 1) * TQB],
                        lhsT=kt[hd * H : (hd + 1) * H, r * TKC : (r + 1) * TKC],
                        rhs=qt[hd * H : (hd + 1) * H, j * TQB : (j + 1) * TQB],
                        start=True,
                        stop=True,
                    )

        def exps(bl, g):
            b, j = bl
            for hd in range(NLOC):
                pt_t = p_pt.tile(
                    [128, RG * TQB], bf16,
                    name=f"pt{b}{j}{g}{hd}", tag=f"pt{hd}",
                )
                st[("pt", bl, g % 2, hd)] = pt_t
                nc.scalar.activation(
                    pt_t[:], st[("ss", bl, g % 2, hd)][:], AF.Exp
                )

        def pvs(bl, g, va):
            b, j = bl
            if g == 0:
                st[("pv", bl)] = [
                    ps_pv.tile([VA, TQB], f32, name=f"pv{b}{j}{hd}", tag=f"pv{hd}")
                    for hd in range(NLOC)
                ]
            pv = st[("pv", bl)]
            for hd in range(NLOC):
                for r in range(g * RG, (g + 1) * RG):
                    col0 = (r * NLOC + hd) * VA
                    nc.tensor.matmul(
                        pv[hd][:],
                        lhsT=va[:, col0 : col0 + VA],
                        rhs=st[("pt", bl, g % 2, hd)][:, (r % RG) * TQB : (r % RG + 1) * TQB],
                        start=(g == 0 and r == g * RG),
                        stop=(g == NG - 1 and r == (g + 1) * RG - 1),
                    )

        def norm_a(bl):
            """Evacuate pv -> SBUF (frees the pv PSUM banks for the next
            block's accumulation)."""
            b, j = bl
            pv = st.pop(("pv", bl))
            for hd in range(NLOC):
                a_sb = p_a.tile(
                    [H + 1, TQB], f32, name=f"as{b}{j}{hd}", tag=f"as{hd}"
                )
                nc.vector.tensor_copy(a_sb[:], pv[hd][0 : H + 1, :])
                st[("as", bl, hd)] = a_sb

        def norm_b(bl, hd):
            """Reciprocal of the denominator row (PE-transposed to [128,4] so
            the DVE op is cheap), broadcast to H rows, multiply, stage into
            the pair's AllToAll input. (Custom DVE ops are unusable: this
            toolchain's walrus rejects InstCustomDveAnt.)"""
            b, j = bl
            NTR = TQB // 128  # 4
            a_sb = st.pop(("as", bl, hd))
            trp = ps_mm.tile([128, NTR], f32, name=f"trp{b}{j}{hd}", tag="mm")
            for i in range(NTR):
                nc.tensor.transpose(
                    trp[:, i : i + 1],
                    a_sb[H : H + 1, i * 128 : (i + 1) * 128],
                    onesf[H : H + 1, 0:1],
                )
            rc = p_a.tile([128, NTR], f32, name=f"rc{b}{j}{hd}", tag=f"rc{hd}")
            nc.vector.reciprocal(rc[:], trp[:])
            rowt = ps_mm.tile([1, TQB], f32, name=f"rw{b}{j}{hd}", tag="mm")
            for i in range(NTR):
                nc.tensor.transpose(
                    rowt[:, i * 128 : (i + 1) * 128],
                    rc[:, i : i + 1],
                    idf_sb[:, 0:128],
                )
            rr = p_a.tile([1, TQB], bf16, name=f"rr{b}{j}{hd}", tag=f"rr{hd}")
            nc.vector.tensor_copy(rr[:], rowt[:])
            rep_ps = ps_mm.tile([H, TQB], f32, name=f"rp{b}{j}{hd}", tag="mm")
            nc.tensor.matmul(
                rep_ps[:], lhsT=onesb[0:1, 0:H], rhs=rr[:], start=True, stop=True
            )
            an = p_a.tile([H, TQB], bf16, name=f"an{b}{j}{hd}", tag=f"an{hd}")
            nc.vector.tensor_tensor(an[:], a_sb[0:H, :], rep_ps[:], ALU.mult)
            if dbg is not None:
                r0 = ((b * NTQB + j) * NLOC + hd) * H
                nc.sync.dma_start(dbg[r0 : r0 + H, :], an[:])
            # stage into the pair buffer: rows c*128 + hd*64 + h, cols
            # (j%2)*64 + own-64 of tq. Iteration order (h, c, t) on both
            # sides; SBUF AP keeps the partition dim (h) first.
            half = j % 2
            dst = a2a_in[(b, j // 2)].rearrange("(c s) t -> s c t", s=NW)[
                hd * H : (hd + 1) * H, :, half * OWN : (half + 1) * OWN
            ]
            src = an[:].rearrange("h (c t) -> h c t", t=OWN)
            # SP queue: behind the x-loads (first ~75us), which is fine — the
            # collective chain is trigger-paced after that. (scalar/gpsimd
            # DMA queues corrupt this strided transfer: NaN / wrong layout.)
            nc.sync.dma_start(dst, src)

        def collective(pp):
            nc.gpsimd.collective_compute(
                "AllToAll",
                mybir.AluOpType.bypass,
                replica_groups=GROUPS,
                ins=[a2a_in[pp].opt()],
                outs=[a2a_out[pp].opt()],
            )

        # ===== main schedule =================================================
        load_startup()
        qt0, kt0, va0, pieces0 = make_proj(0)
        # inline: K0, Q0 only -- scores start as soon as they project.
        pieces0[0]()
        pieces0[1]()
        # b0 filler order = consumption order (V0/K1/va03 lead; Q1/Q2 hoisted
        # so each lands >=1 group before its block's first scores).
        fillers.extend(
            [pieces0[i] for i in (2, 4, 3, 7, 5, 6, 8, 10, 9, 13, 11, 12, 14, 15)]
        )

        qt1, kt1, va1, p1 = make_proj(1)
        # b1 consumption order matching its v/k-interleaved DMA arrival
        fillers.extend(
            [p1[i] for i in (2, 0, 3, 5, 4, 6, 8, 7, 9, 11, 10, 12, 1, 13, 14, 15)]
        )

        # EMISSION ORDER IS DATAFLOW: a filler emitted after its consumer's
        # emission silently reads stale data. Pops per (bi, g), hand-paced to
        # DMA arrival while keeping >=1 group of margin before each consumer.
        # bi=0 list: [V0, K1, va03 | - | K2, V1, va47 | V2 | K3, va811 | Q1 |
        #             V3, va1215, Q2 | -]
        POPS = {
            0: [3, 0, 3, 1, 2, 1, 3, 0],
            1: [0, 0, 0, 2, 1, 2, 1, 1],
            2: [0, 0, 0, 1, 1, 1, 1, 1],
            3: [0, 0, 0, 1, 1, 1, 1, 1],
        }
        DEF_POPS = [0, 0, 0, 0, 2, 2, 2, 2]

        qkv = {0: (qt0, kt0, va0), 1: (qt1, kt1, va1)}
        ats_pairs = {}
        for b in range(B):
            for pr in range(2):
                ats_pairs[(b, pr)] = p_a.tile(
                    [128, NCORES * 128], bf16, name=f"ats{b}{pr}", tag=f"ats{pr}"
                )

        NB = len(BLOCKS)
        scores(BLOCKS[0], 0, qt0, kt0)
        exps(BLOCKS[0], 0)
        for bi, bl in enumerate(BLOCKS):
            b, j = bl
            qt, kt, va = qkv[b]
            for g in range(NG):
                # one-group lookahead (crosses block boundaries)
                if g + 1 < NG:
                    scores(bl, g + 1, qt, kt)
                    exps(bl, g + 1)
                elif bi + 1 < NB:
                    nbl = BLOCKS[bi + 1]
                    nqt, nkt, _ = qkv[nbl[0]]
                    scores(nbl, 0, nqt, nkt)
                    exps(nbl, 0)
                if bi > 0:
                    pbl = BLOCKS[bi - 1]
                    if g == 0:
                        norm_b(pbl, 0)
                    elif g == 1:
                        norm_b(pbl, 1)
                    elif g == 2 and pbl[1] % 2 == 1:
                        collective((pbl[0], pbl[1] // 2))
                    elif g == 3 and bi in (4, 6):
                        # ats for the pair exchanged TWO blocks ago (its
                        # collective is long done -> no SP-queue stall)
                        pb, pj = BLOCKS[bi - 3]
                        pr = pj // 2
                        ats_load(pb, pr, ats_pairs[(pb, pr)])
                        fillers.extend(outproj_pair(pb, pr, ats_pairs[(pb, pr)]))
                run_filler(POPS.get(bi, DEF_POPS)[g])
                pvs(bl, g, va)
                if g == NG - 1:
                    norm_a(bl)

        # ---- tail: last block's norm + pair collectives + outprojs ---------
        last = BLOCKS[-1]
        norm_b(last, 0)
        norm_b(last, 1)
        collective((1, 1))
        # pair (1,0)'s collective was triggered two blocks ago; its outproj
        # runs while pair (1,1)'s collective is in flight.
        ats_load(1, 0, ats_pairs[(1, 0)])
        for p in outproj_pair(1, 0, ats_pairs[(1, 0)]):
            p()
        run_filler(len(fillers))
        if WARMUP:
            # keep the PE HAM-warm through the final collective's PE-idle
            # window so the last outproj doesn't run at 1.2 GHz.
            kwps = ps_mm.tile([64, 64], f32, name="kwps", tag="mm")
            for _ in range(180):
                nc.tensor.matmul(
                    kwps[:], lhsT=wtile[:, 0:64], rhs=wtile[:], start=True, stop=True
                )
        ats_load(1, 1, ats_pairs[(1, 1)])
        for p in outproj_pair(1, 1, ats_pairs[(1, 1)]):
            p()
        if dbg2 is not None:
            for b in range(B):
                for pr in range(2):
                    nc.sync.dma_start(
                        dbg2[(b * 2 + pr) * 128 : (b * 2 + pr + 1) * 128, :],
                        ats_pairs[(b, pr)][:],
                    )

    orig_to_json = nc.to_json_bytes
    nc.to_json_bytes = lambda: _legalize_waits(orig_to_json())
    return nc


def _get_nc():
    if "nc" not in _CACHE:
        _CACHE["nc"] = _build()
    return _CACHE["nc"]


def _make_in_maps(inputs):
    q = np.asarray(inputs["q"], dtype=np.float32)
    v = np.asarray(inputs["v"], dtype=np.float32)
    k = np.asarray(inputs["k"], dtype=np.float32)
    w_query = np.asarray(inputs["w_query"], dtype=np.float32)
    b_query = np.asarray(inputs["b_query"], dtype=np.float32)
    w_value = np.asarray(inputs["w_value"], dtype=np.float32)
    b_value = np.asarray(inputs["b_value"], dtype=np.float32)
    w_key = np.asarray(inputs["w_key"], dtype=np.float32)
    b_key = np.asarray(inputs["b_key"], dtype=np.float32)
    w_projection = np.asarray(inputs["w_projection"], dtype=np.float32)
    b_projection = np.asarray(inputs["b_projection"], dtype=np.float32)

    scale = np.float32(1.0 / np.sqrt(H))

    def arrange_w(w):
        # [D or N*H, m] -> SBUF layout [128, (chunk, m)], contiguous rows
        m = w.shape[1]
        return np.ascontiguousarray(
            w.reshape(-1, 128, m).transpose(1, 0, 2).reshape(128, -1)
        ).astype(BF16)

    def arrange_x(xb):
        # [T, D] -> [D, T] -> SBUF layout [128, (tb, dc, t)], contiguous rows
        return np.ascontiguousarray(
            xb.T.reshape(NDC, 128, NTQB, XB)
            .transpose(1, 2, 0, 3)
            .reshape(128, NTQB * NDC * XB)
        ).astype(BF16)

    wp_s = arrange_w(
        np.ascontiguousarray(
            w_projection.transpose(0, 2, 1).reshape(N_HEADS * H, D)
        )
    )
    bp_s = np.ascontiguousarray(
        np.tile(b_projection.reshape(1, D), (128, 1))
    ).astype(np.float32)

    xT = {}
    for b in range(B):
        xT[b] = tuple(arrange_x(x[b]) for x in (q, k, v))

    in_maps = []
    for c in range(NCORES):
        hs = c * NLOC
        wq_s = arrange_w(w_query[:, hs : hs + NLOC, :].reshape(D, NW) * scale)
        wk_s = arrange_w(w_key[:, hs : hs + NLOC, :].reshape(D, NW))
        wv_s = arrange_w(w_value[:, hs : hs + NLOC, :].reshape(D, NW))
        bq_s = np.ascontiguousarray(
            (b_query[hs : hs + NLOC].reshape(NW) * scale).reshape(NW, 1)
        ).astype(np.float32)
        bk_s = np.ascontiguousarray(
            b_key[hs : hs + NLOC].reshape(NW, 1)
        ).astype(np.float32)
        bv_s = np.ascontiguousarray(
            b_value[hs : hs + NLOC].reshape(NW, 1)
        ).astype(np.float32)
        m = {
            "ident": np.eye(128, dtype=np.float32).astype(BF16),
            "identf": np.eye(128, dtype=np.float32),
            "wq": np.ascontiguousarray(wq_s),
            "wk": np.ascontiguousarray(wk_s),
            "wv": np.ascontiguousarray(wv_s),
            "wp": wp_s,
            "bq": bq_s,
            "bk": bk_s,
            "bv": bv_s,
            "bp": bp_s,
        }
        for b in range(B):
            m[f"qT{b}"], m[f"kT{b}"], m[f"vT{b}"] = xT[b]
        in_maps.append(m)
    return in_maps


def _assemble(results):
    out = np.empty((B, T, D), np.float32)
    for c in range(NCORES):
        res = results[c]["out"]  # [B*2*128, D]: rows (b, pair, i)
        for b in range(B):
            for pr in range(2):
                blk = res[(b * 2 + pr) * 128 : (b * 2 + pr + 1) * 128]
                for half in range(2):
                    j = 2 * pr + half
                    r0 = j * TQB + c * OWN
                    out[b, r0 : r0 + OWN, :] = blk[half * OWN : (half + 1) * OWN]
    return out


def run(inputs, trace=False, **kwargs):
    from concourse.bass_utils import run_bass_kernel_spmd

    nc = _get_nc()
    in_maps = _make_in_maps(inputs)
    res = run_bass_kernel_spmd(
        nc, in_maps, list(range(NCORES)), trace=trace, **kwargs
    )
    return _assemble(res.results), res


def kernel(**inputs) -> np.ndarray:
    out, _ = run(inputs, trace=False)
    return out
